# revision 1
# baseline (speedup 1.0000x reference)
# Trainium2 Bass kernel for nn_EnhancedEURLTransformer_87694642249910
# Sharding: 8 cores = 2 (batch) x 4 (sequence rows). Per-layer AllGather of x
# within each 4-core group. Activations transposed [D on partitions, rows free].
import os
import sys

sys.path.insert(0, "/opt/trn_rl_repo")

import math
import numpy as np
import ml_dtypes

import concourse.bass as bass
import concourse.mybir as mybir
import concourse.tile as tile
from concourse import bacc
from concourse.bass_utils import run_bass_kernel_spmd
from concourse.masks import make_identity

B, S, D, H, R, L, V, FF = 2, 2048, 512, 8, 64, 6, 32000, 2048
HD = D // H          # 64
K_TOP = 409
LN_EPS = 1e-5
P = 128
DK = D // P          # 4 d-chunks
SC = S // P          # 16 seq-chunks
NCORE = 8
RS = S // 4          # 512 rows per core
QC = RS // P         # 4 own-row chunks
VSL = V // 4         # 8000 vocab cols per core
FFC = FF // P        # 16
SQRT_D = math.sqrt(D)

F32 = mybir.dt.float32
F32R = mybir.dt.float32r
BF16 = mybir.dt.bfloat16
I16 = mybir.dt.int16
I32 = mybir.dt.int32
AF = mybir.ActivationFunctionType
OP = mybir.AluOpType
AX = mybir.AxisListType

N_BISECT = int(os.environ.get("K_BISECT", "12"))
L_RUN = int(os.environ.get("K_LAYERS", str(L)))
EN_SPARSE = os.environ.get("K_SPARSE", "1") == "1"
EN_DENSE = os.environ.get("K_DENSE", "1") == "1"
EN_FFN = os.environ.get("K_FFN", "1") == "1"

_CACHE = {}


def _wrap_idx(idx):
    # dma_gather index wrapping: token i -> partition i%16, col i//16
    # tile must be [128, n//16]; only partitions 0..15 are read
    n = idx.shape[0]
    return np.ascontiguousarray(
        np.tile(idx.reshape(n // 16, 16).T.astype(np.int16), (8, 1)))


def build_nc(dump_x=False):
    nc = bacc.Bacc("TRN2", target_bir_lowering=False, debug=False, num_devices=NCORE)

    emb_d = nc.dram_tensor("emb", [V, D], F32, kind="ExternalInput")
    idxo_d = nc.dram_tensor("idxo", [128, RS // 16], I16, kind="ExternalInput")
    peTo_d = nc.dram_tensor("peTo", [P, DK, RS], F32, kind="ExternalInput")
    lam_d = nc.dram_tensor("lam", [1, L], F32R, kind="ExternalInput")
    wl_d = nc.dram_tensor("wl", [L, P, DK, 1152], BF16, kind="ExternalInput")
    wr_d = nc.dram_tensor("wr", [L, P, DK, 1024], BF16, kind="ExternalInput")
    ow_d = nc.dram_tensor("ow", [L, P, DK, D], BF16, kind="ExternalInput")
    f1_d = nc.dram_tensor("f1", [L, P, DK, FF], BF16, kind="ExternalInput")
    f2_d = nc.dram_tensor("f2", [L, P, FFC, D], BF16, kind="ExternalInput")
    lns_d = nc.dram_tensor("lns", [L, 2, P, DK], F32, kind="ExternalInput")
    lnb_d = nc.dram_tensor("lnb", [L, 2, P, DK], F32, kind="ExternalInput")
    fin_d = nc.dram_tensor("fin", [P, DK, VSL], BF16, kind="ExternalInput")
    out_d = nc.dram_tensor("out", [VSL, S], F32, kind="ExternalOutput")
    if dump_x:
        dbg_d = nc.dram_tensor("dbg", [L + 1, P, DK, RS], F32, kind="ExternalOutput")

    from contextlib import ExitStack
    with tile.TileContext(nc) as tc, ExitStack() as ctx:
        ep = ctx.enter_context
        st = ep(tc.tile_pool(name="state", bufs=1))
        sm = ep(tc.tile_pool(name="small", bufs=2))
        finp = ep(tc.tile_pool(name="finp", bufs=2))
        psc = ep(tc.tile_pool(name="psc", bufs=2, space="PSUM"))
        pspv = ep(tc.tile_pool(name="pspv", bufs=1, space="PSUM"))
        phold = ep(tc.tile_pool(name="phold", bufs=2, space="PSUM"))
        dram = ep(tc.tile_pool(name="dram", bufs=1, space="DRAM"))
        ctx2 = ctx.enter_context(ExitStack())
        ep2 = ctx2.enter_context
        wp = ep2(tc.tile_pool(name="wproj", bufs=1))
        wf = ep2(tc.tile_pool(name="wffn", bufs=2))
        wf2 = ep2(tc.tile_pool(name="wf2", bufs=1))
        kv = ep2(tc.tile_pool(name="kv", bufs=1))
        sel = ep2(tc.tile_pool(name="sel", bufs=1))
        sel2 = ep2(tc.tile_pool(name="sel2", bufs=1))
        spt_pool = ep2(tc.tile_pool(name="spTp", bufs=1))
        expp = ep2(tc.tile_pool(name="expp", bufs=2))
        lnp = ep2(tc.tile_pool(name="lnp", bufs=1))
        if True:
            def sc_tile():
                return psc.tile([P, 1024], F32, tag="sc", name="sc")

            def sc_tile_b():
                return psc.tile([P, 1024], BF16, tag="sc", name="scb")

            def hold_tile():
                return phold.tile([P, 512], F32, tag="hold", name="hold")

            # ------------- persistent state -------------
            xT = st.tile([P, DK, S], BF16)          # gathered x (all rows)
            xo = st.tile([P, DK, RS], F32R)         # own rows, residual spine
            ident = st.tile([P, P], F32)
            identb = st.tile([P, P], BF16)
            ones_f32r = st.tile([P, 1], F32R)
            ones_1 = st.tile([1, P], F32R)
            g_all = st.tile([P, L], F32)
            gm_all = st.tile([P, L], F32)
            make_identity(nc, ident)
            make_identity(nc, identb)
            ones_tmp = sm.tile([P, 1], F32, tag="otmp")
            nc.vector.memset(ones_tmp[:], 1.0)
            nc.vector.tensor_copy(ones_f32r[:], ones_tmp[:])
            ones_tmp2 = sm.tile([1, P], F32, tag="otmp2")
            nc.vector.memset(ones_tmp2[:], 1.0)
            nc.vector.tensor_copy(ones_1[:], ones_tmp2[:])

            eps_t = st.tile([1, 1], F32)
            nc.vector.memset(eps_t[:], LN_EPS)
            lam_s = sm.tile([1, L], F32R, tag="lam")
            nc.sync.dma_start(lam_s[:], lam_d.ap())
            g_row = sm.tile([1, L], F32R, tag="lam")
            nc.scalar.activation(g_row[:], lam_s[:], AF.Sigmoid)
            pg = sc_tile()
            nc.tensor.matmul(pg[:, :L], ones_1[:], g_row[:], start=True, stop=True)
            nc.vector.tensor_copy(g_all[:], pg[:, :L])
            nc.vector.tensor_scalar(gm_all[:], g_all[:], -1.0, 1.0,
                                    op0=OP.mult, op1=OP.add)  # 1-g

            ag_in = dram.tile([P, DK, RS], BF16)
            ag_out = dram.tile([4, P, DK, RS], BF16)

            # ---------- embedding: own rows ----------
            idx_s = sm.tile([128, RS // 16], I16, tag="idx")
            nc.sync.dma_start(idx_s[:], idxo_d.ap())
            gath = sel.tile([P, QC, D], F32, tag="E", name="gath")
            nc.gpsimd.dma_gather(gath[:], emb_d.ap(), idx_s[:], RS, RS, D)
            for kk in range(DK):
                pt = sc_tile()
                for c in range(QC):
                    nc.tensor.transpose(pt[:, c * P:(c + 1) * P],
                                        gath[:, c, kk * P:(kk + 1) * P], ident[:])
                nc.vector.tensor_scalar_mul(xo[:, kk, :].bitcast(F32), pt[:, :RS],
                                            SQRT_D)
            nc.gpsimd.dma_start(xo[:].bitcast(F32), peTo_d.ap(),
                                accum_op=OP.add)

            def allgather_x(xo_bf):
                nc.sync.dma_start(ag_in[:], xo_bf[:])
                nc.gpsimd.collective_compute(
                    "AllGather", OP.bypass,
                    replica_groups=[[0, 1, 2, 3], [4, 5, 6, 7]],
                    ins=[ag_in[:].opt()], outs=[ag_out[:].opt()])
                for rr in range(4):
                    nc.sync.dma_start(xT[:, :, rr * RS:(rr + 1) * RS], ag_out[rr])

            if dump_x:
                nc.sync.dma_start(dbg_d.ap()[0], xo[:].bitcast(F32))
            xoB_cur = kv.tile([P, DK, RS], BF16, tag="xoB", name="xoB0")
            nc.vector.tensor_copy(xoB_cur[:], xo[:])
            allgather_x(xoB_cur)

            rs_t = lnp.tile([128, RS], F32, name="rowscratch")
            rs2_t = lnp.tile([1, 3 * RS], F32R, name="rowscratch2")
            # ---------- layernorm: dst = LN(u) ----------
            def layernorm(u, l, which, dst):
                usq_t = []
                for kk in range(DK):
                    usq = lnp.tile([P, RS], F32R, tag="usq", name="usq")
                    nc.vector.tensor_tensor(usq, u[:, kk, :], u[:, kk, :], OP.mult)
                    usq_t.append(usq)
                psum_s = hold_tile()
                for kk in range(DK):
                    nc.tensor.matmul(psum_s[:1, :RS], ones_f32r[:], u[:, kk, :],
                                     start=(kk == 0), stop=(kk == DK - 1))
                mean = rs_t[0:1, :]
                nc.vector.tensor_scalar_mul(mean[:], psum_s[:1, :RS], 1.0 / D)
                pssq = hold_tile()
                for kk in range(DK):
                    nc.tensor.matmul(pssq[:1, :RS], ones_f32r[:], usq_t[kk][:],
                                     start=(kk == 0), stop=(kk == DK - 1))
                msq = rs_t[64:65, :]
                nc.vector.tensor_tensor(msq[:], mean[:], mean[:], OP.mult)
                var = rs_t[32:33, :]
                nc.vector.scalar_tensor_tensor(var[:], pssq[:1, :RS], 1.0 / D,
                                               msq[:], op0=OP.mult,
                                               op1=OP.subtract)
                sd = rs_t[96:97, :]
                nc.scalar.activation(sd[:], var[:], AF.Sqrt, bias=eps_t[:])
                istd = rs2_t[0:1, 0:RS]
                with nc.allow_low_precision(reason="f32r istd"):
                    nc.vector.reciprocal(istd[:], sd[:])
                nistd = rs2_t[0:1, RS:2 * RS]
                nc.vector.tensor_tensor(nistd[:], mean[:].bitcast(F32R), istd[:],
                                        OP.mult)
                nc.vector.tensor_scalar_mul(nistd[:], nistd[:], -1.0)
                pA = hold_tile()
                nc.tensor.matmul(pA[:, :RS], ones_1[:], istd[:], start=True, stop=True)
                pB = hold_tile()
                nc.tensor.matmul(pB[:, :RS], ones_1[:], nistd[:], start=True, stop=True)
                scl = sm.tile([P, DK], F32, tag="ln_sc")
                bcl = sm.tile([P, DK], F32, tag="ln_bc")
                nc.sync.dma_start(scl[:], lns_d.ap()[l, which])
                nc.sync.dma_start(bcl[:], lnb_d.ap()[l, which])
                for kk in range(DK):
                    t0 = dst[:, kk, :]
                    nc.vector.tensor_tensor(t0, u[:, kk, :],
                                            pA[:, :RS].bitcast(F32R), OP.mult)
                    nc.vector.tensor_tensor(t0, t0, pB[:, :RS].bitcast(F32R), OP.add)
                    nc.vector.tensor_scalar(
                        t0, t0, scl[:, kk:kk + 1], bcl[:, kk:kk + 1],
                        op0=OP.mult, op1=OP.add)

            # ================= layers =================
            for l in range(L_RUN):
                wl_s = wp.tile([P, DK, 1152], BF16, tag="wl")
                wr_s = wp.tile([P, DK, 1024], BF16, tag="wr")
                nc.gpsimd.dma_start(wl_s[:], wl_d.ap()[l])
                nc.gpsimd.dma_start(wr_s[:], wr_d.ap()[l])

                # ---- q^T, Qs^T from own rows ----
                qT = kv.tile([P, DK, RS], BF16, tag="qT")
                QsT = kv.tile([64, RS], BF16, tag="QsT")
                xoB = xoB_cur
                for oc in range(DK):
                    pq = sc_tile()
                    for kk in range(DK):
                        nc.tensor.matmul(pq[:, :RS],
                                         wl_s[:, kk, 576 + oc * P:576 + (oc + 1) * P],
                                         xoB[:, kk, :],
                                         start=(kk == 0), stop=(kk == DK - 1))
                    nc.scalar.copy(qT[:, oc, :], pq[:, :RS])
                pq = sc_tile()
                for kk in range(DK):
                    nc.tensor.matmul(pq[:64, :RS], wl_s[:, kk, 1088:1152],
                                     xoB[:, kk, :], start=(kk == 0), stop=(kk == DK - 1))
                nc.scalar.copy(QsT[:], pq[:64, :RS])

                # ---- k^T, Ks^T (full seq) ----
                kT = kv.tile([P, DK, S], BF16, tag="kT")
                KsT = kv.tile([64, S], BF16, tag="KsT")
                for oc in range(DK):
                    for fc in range(S // 1024):
                        pk = sc_tile()
                        for hh in range(2):
                            for kk in range(DK):
                                nc.tensor.matmul(
                                    pk[:, hh * 512:(hh + 1) * 512],
                                    wl_s[:, kk, oc * P:(oc + 1) * P],
                                    xT[:, kk, fc * 1024 + hh * 512:fc * 1024 + (hh + 1) * 512],
                                    start=(kk == 0), stop=(kk == DK - 1))
                        if oc % 2 == 0:
                            nc.vector.tensor_copy(
                                kT[:, oc, fc * 1024:(fc + 1) * 1024], pk[:])
                        else:
                            nc.scalar.copy(kT[:, oc, fc * 1024:(fc + 1) * 1024], pk[:])
                for fc in range(S // 1024):
                    pk = sc_tile()
                    for hh in range(2):
                        for kk in range(DK):
                            nc.tensor.matmul(
                                pk[:64, hh * 512:(hh + 1) * 512],
                                wl_s[:, kk, 512:576],
                                xT[:, kk, fc * 1024 + hh * 512:fc * 1024 + (hh + 1) * 512],
                                start=(kk == 0), stop=(kk == DK - 1))
                    nc.scalar.copy(KsT[:, fc * 1024:(fc + 1) * 1024], pk[:64, :])

                # ---- v520 (ones col per head), Vs ----
                v520 = kv.tile([P, SC, 8 * 65], BF16, tag="v520")
                Vs = kv.tile([P, SC, D], BF16, tag="Vs")
                if l == 0:
                    nc.vector.memset(
                        v520[:].rearrange("p s (h c) -> p s h c", c=65)[:, :, :, 64:],
                        1.0)
                for scn in range(SC):
                    pv_ = sc_tile()
                    for kk in range(DK):
                        nc.tensor.matmul(pv_[:, :512],
                                         xT[:, kk, scn * P:(scn + 1) * P],
                                         wr_s[:, kk, 0:512],
                                         start=(kk == 0), stop=(kk == DK - 1))
                    nc.vector.tensor_copy(
                        v520[:, scn, :].rearrange("p (h c) -> p h c", c=65)[:, :, :64],
                        pv_[:, :512].rearrange("p (h c) -> p h c", c=64))
                    pv2 = sc_tile()
                    for kk in range(DK):
                        nc.tensor.matmul(pv2[:, :512],
                                         xT[:, kk, scn * P:(scn + 1) * P],
                                         wr_s[:, kk, 512:1024],
                                         start=(kk == 0), stop=(kk == DK - 1))
                    nc.scalar.copy(Vs[:, scn, :], pv2[:, :512])

                # ---- sparse path: per q-chunk threshold + mask ----
                for qi in range(QC if EN_SPARSE else 0):
                    E = sel.tile([P, S], F32, tag="E")
                    for half in range(2):
                        pq_ = sc_tile()
                        nc.tensor.matmul(pq_[:, :512],
                                         QsT[:, qi * P:(qi + 1) * P],
                                         KsT[:, half * 1024:half * 1024 + 512],
                                         start=True, stop=True)
                        nc.tensor.matmul(pq_[:, 512:],
                                         QsT[:, qi * P:(qi + 1) * P],
                                         KsT[:, half * 1024 + 512:(half + 1) * 1024],
                                         start=True, stop=True)
                        nc.scalar.activation(E[:, half * 1024:(half + 1) * 1024],
                                             pq_[:], AF.Exp, scale=0.125)
                    Eb = sel2.tile([P, S], BF16, tag="Eb")
                    nc.vector.tensor_copy(Eb[:], E[:])
                    mx = sm.tile([P, 1], F32, tag="se_mx")
                    nc.vector.tensor_reduce(mx[:], E[:], AX.X, OP.max)
                    lo_f = sm.tile([P, 1], F32, tag="se_lo")
                    hi_f = sm.tile([P, 1], F32, tag="se_hi")
                    c_lo = sm.tile([P, 1], F32, tag="se_cl")
                    c_hi = sm.tile([P, 1], F32, tag="se_ch")
                    nc.vector.tensor_scalar_mul(lo_f[:], mx[:], 1.125e-7)  # e^-16
                    nc.vector.tensor_copy(hi_f[:], mx[:])
                    nc.vector.memset(c_lo[:], float(S))
                    nc.vector.memset(c_hi[:], 1.0)
                    junk = sel2.tile([P, S], BF16, tag="spn", name="junk")
                    for it in range(N_BISECT):
                        t_f = sm.tile([P, 1], F32, tag="se_ti")
                        nc.vector.tensor_tensor(t_f[:].bitcast(I32),
                                                lo_f[:].bitcast(I32),
                                                hi_f[:].bitcast(I32), OP.add)
                        nc.vector.tensor_scalar(t_f[:].bitcast(I32),
                                                t_f[:].bitcast(I32), 1, None,
                                                op0=OP.logical_shift_right)
                        cnt = sm.tile([P, 1], F32, tag="se_cnt")
                        nc.vector.tensor_scalar(
                            junk[:], Eb[:], t_f[:], 0.0,
                            op0=OP.is_ge, op1=OP.add, accum_out=cnt[:])
                        ge = sm.tile([P, 1], I32, tag="se_ge")
                        lt = sm.tile([P, 1], I32, tag="se_lt")
                        nc.vector.tensor_scalar(ge[:], cnt[:], float(K_TOP), None,
                                                op0=OP.is_ge)
                        nc.vector.tensor_scalar(lt[:], cnt[:], float(K_TOP), None,
                                                op0=OP.is_lt)
                        nc.vector.copy_predicated(lo_f[:], ge[:], t_f[:])
                        nc.vector.copy_predicated(c_lo[:], ge[:], cnt[:])
                        nc.vector.copy_predicated(hi_f[:], lt[:], t_f[:])
                        nc.vector.copy_predicated(c_hi[:], lt[:], cnt[:])
                    # pick side with count closest to K_TOP (t_f := chosen)
                    dlo = sm.tile([P, 1], F32, tag="se_dlo")
                    dhi = sm.tile([P, 1], F32, tag="se_dhi")
                    nc.vector.tensor_scalar(dlo[:], c_lo[:], float(K_TOP), None,
                                            op0=OP.subtract)
                    nc.vector.tensor_scalar(dhi[:], c_hi[:], -1.0, float(K_TOP),
                                            op0=OP.mult, op1=OP.add)
                    use_lo = sm.tile([P, 1], I32, tag="se_ul")
                    nc.vector.tensor_tensor(use_lo[:], dlo[:], dhi[:], OP.is_le)
                    t_f = sm.tile([P, 1], F32, tag="se_tf")
                    nc.vector.tensor_copy(t_f[:], hi_f[:])
                    nc.vector.copy_predicated(t_f[:], use_lo[:], lo_f[:])
                    ssel = sm.tile([P, 1], F32, tag="se_ss")
                    masked = sel2.tile([P, S], BF16, tag="Eb")
                    nc.vector.scalar_tensor_tensor(masked[:], E[:], t_f[:], E[:],
                                                   op0=OP.is_ge, op1=OP.mult,
                                                   accum_out=ssel[:])
                    rsel = sm.tile([P, 1], F32, tag="se_rs")
                    nc.vector.reciprocal(rsel[:], ssel[:])
                    spn = sel2.tile([P, S], BF16, tag="spn")
                    nc.vector.tensor_scalar(spn[:], masked[:], rsel[:], None,
                                            op0=OP.mult)
                    # transpose this q-chunk into spT
                    if qi == 0:
                        spT = spt_pool.tile([P, SC, RS], BF16, tag="spT",
                                            name="spT")
                    for sc2 in range(SC // 4):
                        ptb = sc_tile_b()
                        for j in range(4):
                            scn = sc2 * 4 + j
                            nc.tensor.transpose(ptb[:, j * P:(j + 1) * P],
                                                spn[:, scn * P:(scn + 1) * P],
                                                identb[:])
                        nc.vector.tensor_copy(
                            spT[:, sc2 * 4:(sc2 + 1) * 4, qi * P:(qi + 1) * P],
                            ptb[:, :512].rearrange("p (a b) -> p a b", b=P))

                # ---- sparse PV -> sp_sb [D, RS] ----
                sp_sb = kv.tile([P, DK, RS], BF16, tag="sp_sb")
                if not EN_SPARSE:
                    nc.vector.memset(sp_sb[:], 0.0)
                for kk in range(DK if EN_SPARSE else 0):
                    pa = pspv.tile([P, 512], F32, tag="pspv", name="pa")
                    for scn in range(SC):
                        nc.tensor.matmul(pa[:, :RS], Vs[:, scn, kk * P:(kk + 1) * P],
                                         spT[:, scn, :],
                                         start=(scn == 0), stop=(scn == SC - 1))
                    nc.scalar.copy(sp_sb[:, kk, :], pa[:, :RS])

                # ---- dense attention ----
                attnT = kv.tile([P, DK, RS], BF16, tag="attnT")
                if not EN_DENSE:
                    nc.vector.memset(attnT[:], 0.0)
                for hp in range(4 if EN_DENSE else 0):
                    pv_ps = [hold_tile(), hold_tile()]
                    for scn in range(SC):
                        psum_sc = sc_tile()
                        for i, h in enumerate((2 * hp, 2 * hp + 1)):
                            po = 64 * (h % 2)
                            nc.tensor.matmul(
                                psum_sc[:, i * 512:(i + 1) * 512],
                                kT[po:po + 64, h // 2, scn * P:(scn + 1) * P],
                                qT[po:po + 64, h // 2, :],
                                start=True, stop=True)
                        eT = expp.tile([P, 1024], BF16, tag="eT")
                        nc.scalar.activation(eT[:], psum_sc[:], AF.Exp, scale=0.125)
                        for i, h in enumerate((2 * hp, 2 * hp + 1)):
                            nc.tensor.matmul(
                                pv_ps[i][:65, :RS],
                                v520[:, scn, h * 65:(h + 1) * 65],
                                eT[:, i * 512:(i + 1) * 512],
                                start=(scn == 0), stop=(scn == SC - 1))
                    for i, h in enumerate((2 * hp, 2 * hp + 1)):
                        den = rs2_t[0:1, 2 * RS:3 * RS]
                        nc.scalar.copy(den[:], pv_ps[i][64:65, :RS])
                        rden = rs2_t[0:1, 0:RS]
                        with nc.allow_low_precision(reason="f32r rden"):
                            nc.vector.reciprocal(rden[:], den[:])
                        prb = sc_tile()
                        nc.tensor.matmul(prb[:64, :RS], ones_1[:, :64], rden[:],
                                         start=True, stop=True)
                        rb = lnp.tile([64, RS], BF16, tag="dn_rb")
                        nc.scalar.copy(rb[:], prb[:64, :RS])
                        po = 64 * (h % 2)
                        nc.vector.tensor_tensor(attnT[po:po + 64, h // 2, :],
                                                pv_ps[i][:64, :RS], rb[:], OP.mult)

                # ---- out proj + gating -> u1; LN1 -> y ----
                u1 = sel.tile([P, DK, RS], F32R, tag="E", name="u1")
                for kk in range(DK):
                    ow_s = wf.tile([P, DK, P], BF16, tag="ow")
                    nc.gpsimd.dma_start(ow_s[:], ow_d.ap()[l, :, :, kk * P:(kk + 1) * P])
                    pd = hold_tile()
                    for kk2 in range(DK):
                        nc.tensor.matmul(pd[:, :RS], ow_s[:, kk2, :],
                                         attnT[:, kk2, :],
                                         start=(kk2 == 0), stop=(kk2 == DK - 1))
                    nc.vector.scalar_tensor_tensor(
                        u1[:, kk, :], pd[:, :RS], g_all[:, l:l + 1], xo[:, kk, :],
                        op0=OP.mult, op1=OP.add)
                    nc.vector.scalar_tensor_tensor(
                        u1[:, kk, :], sp_sb[:, kk, :], gm_all[:, l:l + 1],
                        u1[:, kk, :], op0=OP.mult, op1=OP.add)
                y = st.tile([P, DK, RS], F32R, tag="y")
                layernorm(u1, l, 0, y)

                # ---- FFN ----
                yB = kv.tile([P, DK, RS], BF16, tag="xoB")
                nc.vector.tensor_copy(yB[:], y[:])
                hT = spt_pool.tile([P, SC, RS], BF16, tag="spT", name="hT")
                if not EN_FFN:
                    nc.vector.memset(hT[:], 0.0)
                for fg in range(4 if EN_FFN else 0):
                    f1_s = wf.tile([P, DK, 4 * P], BF16, tag="f1")
                    nc.gpsimd.dma_start(f1_s[:], f1_d.ap()[l, :, :, fg * 512:(fg + 1) * 512])
                    for j in range(4):
                        oc = fg * 4 + j
                        ph = sc_tile()
                        for kk in range(DK):
                            nc.tensor.matmul(ph[:, :RS],
                                             f1_s[:, kk, j * P:(j + 1) * P],
                                             yB[:, kk, :],
                                             start=(kk == 0), stop=(kk == DK - 1))
                        nc.scalar.activation(hT[:, oc, :], ph[:, :RS], AF.Relu)
                f2_s = wf2.tile([P, FFC, D], BF16, tag="f2")
                nc.gpsimd.dma_start(f2_s[:], f2_d.ap()[l])
                u2 = sel.tile([P, DK, RS], F32R, tag="E", name="u2")
                for kk in range(DK):
                    pf = hold_tile()
                    for oc in range(FFC):
                        nc.tensor.matmul(pf[:, :RS],
                                         f2_s[:, oc, kk * P:(kk + 1) * P],
                                         hT[:, oc, :],
                                         start=(oc == 0), stop=(oc == FFC - 1))
                    nc.vector.tensor_tensor(u2[:, kk, :], pf[:, :RS], y[:, kk, :],
                                            OP.add)
                layernorm(u2, l, 1, xo)
                if dump_x:
                    nc.sync.dma_start(dbg_d.ap()[l + 1], xo[:].bitcast(F32))
                xoB_cur = kv.tile([P, DK, RS], BF16, tag="xoB", name="xoBn")
                nc.vector.tensor_copy(xoB_cur[:], xo[:])
                allgather_x(xoB_cur)

            if os.environ.get("K_PRINT"):
                import contextlib
                with open("/tmp/prog.txt", "w") as f:
                    with contextlib.redirect_stdout(f):
                        nc.print_concise(deps=True)
            ctx2.close()
            # ================= final projection =================
            NVC = (VSL + P - 1) // P
            for vc in range(NVC):
                grp, off = vc // 4, vc % 4
                if off == 0:
                    ftile = finp.tile([P, DK, 4 * P], BF16, tag="fin")
                    w = min(4 * P, VSL - grp * 4 * P)
                    nc.gpsimd.dma_start(ftile[:, :, :w],
                                      fin_d.ap()[:, :, grp * 4 * P:grp * 4 * P + w])
                vw = min(P, VSL - vc * P)
                for rc in range(4):
                    pl = sc_tile()
                    for kk in range(DK):
                        nc.tensor.matmul(
                            pl[:vw, :512], ftile[:, kk, off * P:off * P + vw],
                            xT[:, kk, rc * 512:(rc + 1) * 512],
                            start=(kk == 0), stop=(kk == DK - 1))
                    lo_s = finp.tile([P, 512], F32, tag="lo")
                    if (vc + rc) % 2 == 0:
                        nc.scalar.copy(lo_s[:vw, :], pl[:vw, :512])
                    else:
                        nc.vector.tensor_copy(lo_s[:vw, :], pl[:vw, :512])
                    nc.sync.dma_start(
                        out_d.ap()[vc * P:vc * P + vw, rc * 512:(rc + 1) * 512],
                        lo_s[:vw, :])

    nc.compile()
    return nc


def _prep_inputs(inputs):
    f32 = np.float32
    bf = ml_dtypes.bfloat16
    src = np.asarray(inputs["src"]).astype(np.int64)
    emb = np.ascontiguousarray(np.asarray(inputs["emb"], f32))
    pe = np.asarray(inputs["pe"], f32)
    lam = np.asarray(inputs["lam"], f32)
    for nm in ("in_b", "out_b", "qp_b", "kp_b", "vp_b", "ff1_b", "ff2_b", "fin_b"):
        assert not np.any(np.asarray(inputs[nm])), f"nonzero bias {nm} unsupported"
    in_w = np.asarray(inputs["in_w"], f32)
    out_w = np.asarray(inputs["out_w"], f32)
    qp_w = np.asarray(inputs["qp_w"], f32)
    kp_w = np.asarray(inputs["kp_w"], f32)
    vp_w = np.asarray(inputs["vp_w"], f32)
    ff1_w = np.asarray(inputs["ff1_w"], f32)
    ff2_w = np.asarray(inputs["ff2_w"], f32)
    ln1_s = np.asarray(inputs["ln1_s"], f32)
    ln1_b = np.asarray(inputs["ln1_b"], f32)
    ln2_s = np.asarray(inputs["ln2_s"], f32)
    ln2_b = np.asarray(inputs["ln2_b"], f32)
    fin_w = np.asarray(inputs["fin_w"], f32)

    def to_pdk(w):  # [L, D, C] -> [L, P, DK, C]
        Lx, Dx, Cx = w.shape
        return np.ascontiguousarray(
            w.reshape(Lx, DK, P, Cx).transpose(0, 2, 1, 3))

    wl = to_pdk(np.concatenate([
        in_w[:, 512:1024, :].transpose(0, 2, 1),
        kp_w.transpose(0, 2, 1),
        in_w[:, 0:512, :].transpose(0, 2, 1),
        qp_w.transpose(0, 2, 1),
    ], axis=2)).astype(bf)
    wr = to_pdk(np.concatenate([
        in_w[:, 1024:1536, :].transpose(0, 2, 1),
        vp_w.transpose(0, 2, 1),
    ], axis=2)).astype(bf)
    ow = to_pdk(out_w.transpose(0, 2, 1)).astype(bf)
    f1 = to_pdk(ff1_w.transpose(0, 2, 1)).astype(bf)
    f2 = np.ascontiguousarray(
        ff2_w.transpose(0, 2, 1).reshape(L, FFC, P, D).transpose(0, 2, 1, 3)).astype(bf)
    lns = np.ascontiguousarray(
        np.stack([ln1_s, ln2_s], 1).reshape(L, 2, DK, P).transpose(0, 1, 3, 2))
    lnb = np.ascontiguousarray(
        np.stack([ln1_b, ln2_b], 1).reshape(L, 2, DK, P).transpose(0, 1, 3, 2))
    peT = np.ascontiguousarray(pe.T.reshape(DK, P, S).transpose(1, 0, 2))
    finT = np.ascontiguousarray(fin_w.T.reshape(DK, P, V).transpose(1, 0, 2))

    in_maps = []
    for c in range(NCORE):
        b, r = c // 4, c % 4
        in_maps.append({
            "emb": emb,
            "idxo": _wrap_idx(src[b, r * RS:(r + 1) * RS]),
            "peTo": np.ascontiguousarray(peT[:, :, r * RS:(r + 1) * RS]),
            "lam": lam.reshape(1, L).astype(f32),
            "wl": wl, "wr": wr, "ow": ow, "f1": f1, "f2": f2,
            "lns": lns, "lnb": lnb,
            "fin": np.ascontiguousarray(
                finT[:, :, r * VSL:(r + 1) * VSL]).astype(bf),
        })
    return in_maps


def kernel(**inputs):
    dump_x = bool(int(os.environ.get("KERNEL_DUMP_X", "0")))
    key = ("nc", dump_x)
    if key not in _CACHE:
        _CACHE[key] = build_nc(dump_x)
    nc = _CACHE[key]
    in_maps = _prep_inputs(inputs)
    trace = bool(int(os.environ.get("KERNEL_TRACE", "0")))
    res = run_bass_kernel_spmd(nc, in_maps, core_ids=list(range(NCORE)),
                               trace=trace)
    if trace:
        _CACHE["last_res"] = res
    out = np.zeros((B, S, V), np.float32)
    for c in range(NCORE):
        b, r = c // 4, c % 4
        out[b, :, r * VSL:(r + 1) * VSL] = res.results[c]["out"].T
    if dump_x:
        _CACHE["dbg"] = [res.results[c].get("dbg") for c in range(NCORE)]
    return out



# revision 20
# speedup vs baseline: 1.6899x; 1.6899x over previous
# Trainium2 Bass kernel for nn_EnhancedEURLTransformer_87694642249910
# Sharding: 8 cores = 2 (batch) x 4 (sequence rows). Per-layer AllGather of x
# within each 4-core group (fp8 payload; final AG bf16 for the logit matmul).
# Activations transposed [D on partitions, rows free]. fp8e4 DoubleRow matmuls
# for all D-contraction projections and both PV contractions; FFN / out-proj /
# final projection stay bf16. Sparse top-K threshold found by a warm-started
# bisection seeded from per-row log-normal stats of exp(qk/8).
import os
import sys

sys.path.insert(0, "/opt/trn_rl_repo")

import math
import numpy as np
import ml_dtypes

import concourse.bass as bass
import concourse.mybir as mybir
import concourse.tile as tile
from concourse import bacc
from concourse.bass_utils import run_bass_kernel_spmd
from concourse.masks import make_identity

B, S, D, H, R, L, V, FF = 2, 2048, 512, 8, 64, 6, 32000, 2048
HD = D // H          # 64
K_TOP = 409
LN_EPS = 1e-5

F32 = mybir.dt.float32
F32R = mybir.dt.float32r
BF16 = mybir.dt.bfloat16
F8 = mybir.dt.float8e4
I16 = mybir.dt.int16
I32 = mybir.dt.int32
AF = mybir.ActivationFunctionType
OP = mybir.AluOpType
AX = mybir.AxisListType
DR = mybir.MatmulPerfMode.DoubleRow

P = 128
DK = D // P          # 4 d-chunks
SC = S // P          # 16 seq-chunks
NCORE = 8
RS = S // 4          # 512 rows per core
QC = RS // P         # 4 own-row chunks
VSL = V // 4         # 8000 vocab cols per core
FFC = FF // P        # 16
SQRT_D = math.sqrt(D)
WSCALE = 64.0        # fp8 projection-weight prescale (avoids subnormals)
LN_N = math.log(float(S))
Z_LO, Z_HI = 0.72, 0.97

N_BISECT = int(os.environ.get("K_BISECT", "5"))
L_RUN = int(os.environ.get("K_LAYERS", str(L)))
EN_SPARSE = os.environ.get("K_SPARSE", "1") == "1"
EN_DENSE = os.environ.get("K_DENSE", "1") == "1"
EN_FFN = os.environ.get("K_FFN", "1") == "1"

_CACHE = {}


def _wrap_idx(idx):
    # dma_gather index wrapping: token i -> partition i%16, col i//16
    # tile must be [128, n//16]; only partitions 0..15 are read
    n = idx.shape[0]
    return np.ascontiguousarray(
        np.tile(idx.reshape(n // 16, 16).T.astype(np.int16), (8, 1)))


def build_nc(dump_x=False):
    nc = bacc.Bacc("TRN2", target_bir_lowering=False, debug=False, num_devices=NCORE)

    emb_d = nc.dram_tensor("emb", [V, D], F32, kind="ExternalInput")
    idxo_d = nc.dram_tensor("idxo", [128, RS // 16], I16, kind="ExternalInput")
    peTo_d = nc.dram_tensor("peTo", [P, DK, RS], F32, kind="ExternalInput")
    lam_d = nc.dram_tensor("lam", [1, L], F32R, kind="ExternalInput")
    wl_d = nc.dram_tensor("wl", [L, P, DK, 1152], F8, kind="ExternalInput")
    wr_d = nc.dram_tensor("wr", [L, P, DK, 1024], F8, kind="ExternalInput")
    ow_d = nc.dram_tensor("ow", [L, P, DK, D], BF16, kind="ExternalInput")
    f1_d = nc.dram_tensor("f1", [L, P, DK, FF], BF16, kind="ExternalInput")
    f2_d = nc.dram_tensor("f2", [L, P, FFC, D], BF16, kind="ExternalInput")
    lns_d = nc.dram_tensor("lns", [L, 2, P, DK], F32, kind="ExternalInput")
    lnb_d = nc.dram_tensor("lnb", [L, 2, P, DK], F32, kind="ExternalInput")
    fin_d = nc.dram_tensor("fin", [P, DK, VSL], BF16, kind="ExternalInput")
    out_d = nc.dram_tensor("out", [VSL, S], BF16, kind="ExternalOutput")
    if dump_x:
        dbg_d = nc.dram_tensor("dbg", [L + 1, P, DK, RS], F32, kind="ExternalOutput")

    from contextlib import ExitStack
    with tile.TileContext(nc) as tc, ExitStack() as ctx:
        ep = ctx.enter_context
        st = ep(tc.tile_pool(name="state", bufs=1))
        sm = ep(tc.tile_pool(name="small", bufs=2))
        psc = ep(tc.tile_pool(name="psc", bufs=2, space="PSUM"))
        pspv = ep(tc.tile_pool(name="pspv", bufs=1, space="PSUM"))
        phold = ep(tc.tile_pool(name="phold", bufs=2, space="PSUM"))
        dram = ep(tc.tile_pool(name="dram", bufs=1, space="DRAM"))
        ctx2 = ctx.enter_context(ExitStack())
        ep2 = ctx2.enter_context
        wp = ep2(tc.tile_pool(name="wproj", bufs=1))
        wf = ep2(tc.tile_pool(name="wffn", bufs=2))
        wf2 = ep2(tc.tile_pool(name="wf2", bufs=1))
        kv = ep2(tc.tile_pool(name="kv", bufs=1))
        sel = ep2(tc.tile_pool(name="sel", bufs=1))
        sel2 = ep2(tc.tile_pool(name="sel2", bufs=1))
        spt_pool = ep2(tc.tile_pool(name="spTp", bufs=1))
        expp = ep2(tc.tile_pool(name="expp", bufs=2))
        lnp = ep2(tc.tile_pool(name="lnp", bufs=1))
        if True:
            def sc_tile():
                return psc.tile([P, 1024], F32, tag="sc", name="sc")

            def sc_tile_b():
                return psc.tile([P, 1024], BF16, tag="sc", name="scb")

            def hold_tile():
                return phold.tile([P, 512], F32, tag="hold", name="hold")

            # ------------- persistent state -------------
            xT8 = st.tile([P, DK, S], F8)           # gathered x (all rows, fp8)
            xo = st.tile([P, DK, RS], F32R)         # own rows, residual spine
            ident = st.tile([P, P], F32)
            identb = st.tile([P, P], BF16)
            ones_f32r = st.tile([P, 1], F32R)
            ones_1 = st.tile([1, P], F32R)
            g_all = st.tile([P, L], F32)
            gm_all = st.tile([P, L], F32)
            make_identity(nc, ident)
            make_identity(nc, identb)
            ones_tmp = sm.tile([P, 1], F32, tag="otmp")
            nc.vector.memset(ones_tmp[:], 1.0)
            nc.vector.tensor_copy(ones_f32r[:], ones_tmp[:])
            ones_tmp2 = sm.tile([1, P], F32, tag="otmp2")
            nc.vector.memset(ones_tmp2[:], 1.0)
            nc.vector.tensor_copy(ones_1[:], ones_tmp2[:])

            eps_t = st.tile([1, 1], F32)
            nc.vector.memset(eps_t[:], LN_EPS)
            eps_p = st.tile([P, 1], F32)
            nc.vector.memset(eps_p[:], 1e-6)
            lam_s = sm.tile([1, L], F32R, tag="lam")
            nc.sync.dma_start(lam_s[:], lam_d.ap())
            g_row = sm.tile([1, L], F32R, tag="lam")
            nc.scalar.activation(g_row[:], lam_s[:], AF.Sigmoid)
            pg = sc_tile()
            nc.tensor.matmul(pg[:, :L], ones_1[:], g_row[:], start=True, stop=True)
            nc.vector.tensor_copy(g_all[:], pg[:, :L])
            nc.vector.tensor_scalar(gm_all[:], g_all[:], -1.0, 1.0,
                                    op0=OP.mult, op1=OP.add)  # 1-g

            ag_in8 = dram.tile([P, DK, RS], F8)
            ag_out8 = dram.tile([4, P, DK, RS], F8)

            # ---------- embedding: own rows ----------
            idx_s = sm.tile([128, RS // 16], I16, tag="idx")
            nc.sync.dma_start(idx_s[:], idxo_d.ap())
            gath = sel.tile([P, QC, D], F32, tag="E", name="gath")
            nc.gpsimd.dma_gather(gath[:], emb_d.ap(), idx_s[:], RS, RS, D)
            for kk in range(DK):
                pt = sc_tile()
                for c in range(QC):
                    nc.tensor.transpose(pt[:, c * P:(c + 1) * P],
                                        gath[:, c, kk * P:(kk + 1) * P], ident[:])
                nc.vector.tensor_scalar_mul(xo[:, kk, :].bitcast(F32), pt[:, :RS],
                                            SQRT_D)
            nc.gpsimd.dma_start(xo[:].bitcast(F32), peTo_d.ap(),
                                accum_op=OP.add)

            def allgather_x8(xo8_src):
                nc.sync.dma_start(ag_in8[:], xo8_src[:])
                nc.gpsimd.collective_compute(
                    "AllGather", OP.bypass,
                    replica_groups=[[0, 1, 2, 3], [4, 5, 6, 7]],
                    ins=[ag_in8[:].opt()], outs=[ag_out8[:].opt()])
                for rr in range(4):
                    nc.sync.dma_start(xT8[:, :, rr * RS:(rr + 1) * RS], ag_out8[rr])

            if dump_x:
                nc.sync.dma_start(dbg_d.ap()[0], xo[:].bitcast(F32))
            xo8_cur = kv.tile([P, DK, RS], F8, tag="xo8", name="xo8e")
            nc.vector.tensor_copy(xo8_cur[:], xo[:])
            allgather_x8(xo8_cur)

            rs_t = lnp.tile([128, 3 * RS], F32, name="rowscratch")
            rs2_t = lnp.tile([1, 3 * RS], F32R, name="rowscratch2")
            # ---------- layernorm: dst = LN(u) ----------
            def layernorm(u, l, which, dst):
                usq_t = []
                for kk in range(DK):
                    usq = lnp.tile([P, RS], F32R, tag="usq", name="usq")
                    nc.vector.tensor_tensor(usq, u[:, kk, :], u[:, kk, :], OP.mult)
                    usq_t.append(usq)
                psum_s = hold_tile()
                for kk in range(DK):
                    nc.tensor.matmul(psum_s[:1, :RS], ones_f32r[:], u[:, kk, :],
                                     start=(kk == 0), stop=(kk == DK - 1))
                mean = rs_t[0:1, 0:RS]
                nc.vector.tensor_scalar_mul(mean[:], psum_s[:1, :RS], 1.0 / D)
                pssq = hold_tile()
                for kk in range(DK):
                    nc.tensor.matmul(pssq[:1, :RS], ones_f32r[:], usq_t[kk][:],
                                     start=(kk == 0), stop=(kk == DK - 1))
                msq = rs_t[64:65, 0:RS]
                nc.vector.tensor_tensor(msq[:], mean[:], mean[:], OP.mult)
                var = rs_t[32:33, 0:RS]
                nc.vector.scalar_tensor_tensor(var[:], pssq[:1, :RS], 1.0 / D,
                                               msq[:], op0=OP.mult,
                                               op1=OP.subtract)
                sd = rs_t[0:1, RS:2 * RS]
                nc.scalar.activation(sd[:], var[:], AF.Sqrt, bias=eps_t[:])
                rtmp = rs_t[0:1, 2 * RS:3 * RS]
                with nc.allow_low_precision(reason="f32r istd"):
                    nc.vector.reciprocal_approx_fast(rtmp[:], sd[:])
                istd = rs2_t[0:1, 0:RS]
                nc.vector.tensor_copy(istd[:], rtmp[:])
                nistd = rs2_t[0:1, RS:2 * RS]
                nc.vector.tensor_tensor(nistd[:], mean[:].bitcast(F32R), istd[:],
                                        OP.mult)
                nc.vector.tensor_scalar_mul(nistd[:], nistd[:], -1.0)
                pA = hold_tile()
                nc.tensor.matmul(pA[:, :RS], ones_1[:], istd[:], start=True, stop=True)
                pB = hold_tile()
                nc.tensor.matmul(pB[:, :RS], ones_1[:], nistd[:], start=True, stop=True)
                scl = sm.tile([P, DK], F32, tag="ln_sc")
                bcl = sm.tile([P, DK], F32, tag="ln_bc")
                nc.sync.dma_start(scl[:], lns_d.ap()[l, which])
                nc.sync.dma_start(bcl[:], lnb_d.ap()[l, which])
                for kk in range(DK):
                    t0 = dst[:, kk, :]
                    nc.vector.tensor_tensor(t0, u[:, kk, :],
                                            pA[:, :RS].bitcast(F32R), OP.mult)
                    nc.vector.tensor_tensor(t0, t0, pB[:, :RS].bitcast(F32R), OP.add)
                    nc.vector.tensor_scalar(
                        t0, t0, scl[:, kk:kk + 1], bcl[:, kk:kk + 1],
                        op0=OP.mult, op1=OP.add)

            # ================= layers =================
            for l in range(L_RUN):
                wl_s = wp.tile([P, DK, 1152], F8, tag="wl")
                wr_s = wp.tile([P, DK, 1024], F8, tag="wr")
                nc.gpsimd.dma_start(wl_s[:], wl_d.ap()[l])
                nc.gpsimd.dma_start(wr_s[:], wr_d.ap()[l])
                xo8 = xo8_cur

                # ---- q^T, Qs^T from own rows (fp8 DoubleRow pairs) ----
                qT = kv.tile([P, DK, RS], F8, tag="qT")
                QsT = kv.tile([64, RS], F8, tag="QsT")
                for oc in range(DK):
                    pq = sc_tile()
                    for kk2 in (0, 2):
                        nc.tensor.matmul(pq[:, :RS],
                                         wl_s[:, kk2:kk2 + 2,
                                              576 + oc * P:576 + (oc + 1) * P],
                                         xo8[:, kk2:kk2 + 2, :],
                                         start=(kk2 == 0), stop=(kk2 == 2),
                                         perf_mode=DR)
                    nc.scalar.activation(qT[:, oc, :], pq[:, :RS], AF.Copy,
                                         scale=1.0 / WSCALE)
                pq = sc_tile()
                for kk2 in (0, 2):
                    nc.tensor.matmul(pq[:64, :RS], wl_s[:, kk2:kk2 + 2, 1088:1152],
                                     xo8[:, kk2:kk2 + 2, :],
                                     start=(kk2 == 0), stop=(kk2 == 2),
                                     perf_mode=DR)
                nc.scalar.activation(QsT[:], pq[:64, :RS], AF.Copy,
                                     scale=1.0 / WSCALE)

                # ---- k^T, Ks^T (full seq, fp8 DoubleRow) ----
                kT = kv.tile([P, DK, S], F8, tag="kT")
                KsT = kv.tile([64, S], F8, tag="KsT")
                for oc in range(DK):
                    for fc in range(S // 1024):
                        pk = sc_tile()
                        for hh in range(2):
                            for kk2 in (0, 2):
                                nc.tensor.matmul(
                                    pk[:, hh * 512:(hh + 1) * 512],
                                    wl_s[:, kk2:kk2 + 2, oc * P:(oc + 1) * P],
                                    xT8[:, kk2:kk2 + 2,
                                        fc * 1024 + hh * 512:fc * 1024 + (hh + 1) * 512],
                                    start=(kk2 == 0), stop=(kk2 == 2),
                                    perf_mode=DR)
                        if oc % 2 == 0:
                            nc.vector.tensor_scalar_mul(
                                kT[:, oc, fc * 1024:(fc + 1) * 1024], pk[:],
                                1.0 / WSCALE)
                        else:
                            nc.scalar.activation(
                                kT[:, oc, fc * 1024:(fc + 1) * 1024], pk[:],
                                AF.Copy, scale=1.0 / WSCALE)
                for fc in range(S // 1024):
                    pk = sc_tile()
                    for hh in range(2):
                        for kk2 in (0, 2):
                            nc.tensor.matmul(
                                pk[:64, hh * 512:(hh + 1) * 512],
                                wl_s[:, kk2:kk2 + 2, 512:576],
                                xT8[:, kk2:kk2 + 2,
                                    fc * 1024 + hh * 512:fc * 1024 + (hh + 1) * 512],
                                start=(kk2 == 0), stop=(kk2 == 2),
                                perf_mode=DR)
                    nc.scalar.activation(KsT[:, fc * 1024:(fc + 1) * 1024],
                                         pk[:64, :], AF.Copy, scale=1.0 / WSCALE)

                # ---- sparse E = exp(qk/8) + warm-start stats ----
                # per qi: Eb [P,S] bf16; accums S1 = sum E, S2 = sum E^2
                if EN_SPARSE:
                    Eb_t, lo_t, hi_t = [], [], []
                for qi in range(QC if EN_SPARSE else 0):
                    Eb = sel2.tile([P, S], BF16, tag="Eb", name="Eb", bufs=4)
                    a1 = sm.tile([P, 2], F32, tag="se_a1")
                    a2 = sm.tile([P, 2], F32, tag="se_a2")
                    for half in range(2):
                        pq_ = sc_tile()
                        nc.tensor.matmul(pq_[:, :512],
                                         QsT[:, qi * P:(qi + 1) * P],
                                         KsT[:, half * 1024:half * 1024 + 512],
                                         start=True, stop=True)
                        nc.tensor.matmul(pq_[:, 512:],
                                         QsT[:, qi * P:(qi + 1) * P],
                                         KsT[:, half * 1024 + 512:(half + 1) * 1024],
                                         start=True, stop=True)
                        nc.scalar.activation(Eb[:, half * 1024:(half + 1) * 1024],
                                             pq_[:], AF.Exp, scale=0.125,
                                             accum_out=a1[:, half:half + 1])
                        junk2 = sel2.tile([P, S], BF16, tag="jk", name="jk2",
                                          bufs=2)
                        nc.scalar.activation(junk2[:, :1024], pq_[:], AF.Exp,
                                             scale=0.25,
                                             accum_out=a2[:, half:half + 1])
                    # log-normal stats -> bisect bracket [lo, hi]
                    s1 = sm.tile([P, 1], F32, tag="se_s1")
                    s2m = sm.tile([P, 1], F32, tag="se_s2")
                    nc.vector.tensor_tensor(s1[:], a1[:, 0:1], a1[:, 1:2], OP.add)
                    nc.vector.tensor_tensor(s2m[:], a2[:, 0:1], a2[:, 1:2], OP.add)
                    L1 = sm.tile([P, 1], F32, tag="se_l1")
                    L2 = sm.tile([P, 1], F32, tag="se_l2")
                    nc.scalar.activation(L1[:], s1[:], AF.Ln)
                    nc.scalar.activation(L2[:], s2m[:], AF.Ln)
                    sig2 = sm.tile([P, 1], F32, tag="se_sg2")
                    nc.vector.scalar_tensor_tensor(sig2[:], L1[:], -2.0, L2[:],
                                                   op0=OP.mult, op1=OP.add)
                    nc.vector.tensor_scalar(sig2[:], sig2[:], 1.0, LN_N,
                                            op0=OP.mult, op1=OP.add)
                    sig = sm.tile([P, 1], F32, tag="se_sg")
                    nc.scalar.activation(sig[:], sig2[:], AF.Sqrt, bias=eps_p[:])
                    mu = sm.tile([P, 1], F32, tag="se_mu")
                    nc.vector.tensor_scalar(mu[:], sig2[:], -0.5, -LN_N,
                                            op0=OP.mult, op1=OP.add)
                    nc.vector.tensor_tensor(mu[:], mu[:], L1[:], OP.add)
                    ulo = sm.tile([P, 1], F32, tag="se_ulo")
                    uhi = sm.tile([P, 1], F32, tag="se_uhi")
                    nc.vector.scalar_tensor_tensor(ulo[:], sig[:], Z_LO, mu[:],
                                                   op0=OP.mult, op1=OP.add)
                    nc.vector.scalar_tensor_tensor(uhi[:], sig[:], Z_HI, mu[:],
                                                   op0=OP.mult, op1=OP.add)
                    lo_f = sm.tile([P, 1], F32, tag="se_lo", name=f"lo{qi}",
                                   bufs=4)
                    hi_f = sm.tile([P, 1], F32, tag="se_hi", name=f"hi{qi}",
                                   bufs=4)
                    nc.scalar.activation(lo_f[:], ulo[:], AF.Exp)
                    nc.scalar.activation(hi_f[:], uhi[:], AF.Exp)
                    Eb_t.append(Eb)
                    lo_t.append(lo_f)
                    hi_t.append(hi_f)

                # ---- v520 (ones col per head), Vs (fp8 DoubleRow) ----
                v520 = kv.tile([P, SC, 8 * 80], F8, tag="v520")
                Vs = kv.tile([P, SC, D], F8, tag="Vs")
                if l == 0:
                    nc.vector.memset(
                        v520[:].rearrange("p s (h c) -> p s h c", c=80)[:, :, :, 64:65],
                        1.0)
                for scn in range(SC):
                    pv_ = sc_tile()
                    for kk2 in (0, 2):
                        nc.tensor.matmul(pv_[:, :512],
                                         xT8[:, kk2:kk2 + 2, scn * P:(scn + 1) * P],
                                         wr_s[:, kk2:kk2 + 2, 0:512],
                                         start=(kk2 == 0), stop=(kk2 == 2),
                                         perf_mode=DR)
                    nc.vector.tensor_scalar_mul(
                        v520[:, scn, :].rearrange("p (h c) -> p h c", c=80)[:, :, :64],
                        pv_[:, :512].rearrange("p (h c) -> p h c", c=64),
                        1.0 / WSCALE)
                    pv2 = sc_tile()
                    for kk2 in (0, 2):
                        nc.tensor.matmul(pv2[:, :512],
                                         xT8[:, kk2:kk2 + 2, scn * P:(scn + 1) * P],
                                         wr_s[:, kk2:kk2 + 2, 512:1024],
                                         start=(kk2 == 0), stop=(kk2 == 2),
                                         perf_mode=DR)
                    nc.scalar.activation(Vs[:, scn, :], pv2[:, :512], AF.Copy,
                                         scale=1.0 / WSCALE)

                # ---- dense attention (scores fp8 K=64; PV fp8 DoubleRow) ----
                attnT = kv.tile([P, DK, RS], BF16, tag="attnT")
                if not EN_DENSE:
                    nc.vector.memset(attnT[:], 0.0)
                for hp in range(4 if EN_DENSE else 0):
                    pv_ps = [hold_tile(), hold_tile()]
                    for scp in range(SC // 2):
                        eTp = expp.tile([P, 2, 1024], F8, tag="eT")
                        for j in range(2):
                            scn = 2 * scp + j
                            psum_sc = sc_tile()
                            for i, h in enumerate((2 * hp, 2 * hp + 1)):
                                po = 64 * (h % 2)
                                nc.tensor.matmul(
                                    psum_sc[:, i * 512:(i + 1) * 512],
                                    kT[po:po + 64, h // 2, scn * P:(scn + 1) * P],
                                    qT[po:po + 64, h // 2, :],
                                    start=True, stop=True)
                            nc.scalar.activation(eTp[:, j, :], psum_sc[:], AF.Exp,
                                                 scale=0.125)
                        for i, h in enumerate((2 * hp, 2 * hp + 1)):
                            nc.tensor.matmul(
                                pv_ps[i][:65, :RS],
                                v520[:, 2 * scp:2 * scp + 2, h * 80:h * 80 + 65],
                                eTp[:, :, i * 512:(i + 1) * 512],
                                start=(scp == 0), stop=(scp == SC // 2 - 1),
                                perf_mode=DR)
                    for i, h in enumerate((2 * hp, 2 * hp + 1)):
                        den = rs_t[0:1, RS:2 * RS]
                        nc.scalar.copy(den[:], pv_ps[i][64:65, :RS])
                        rtmp = rs_t[0:1, 2 * RS:3 * RS]
                        with nc.allow_low_precision(reason="f32r rden"):
                            nc.vector.reciprocal_approx_fast(rtmp[:], den[:])
                        rden = rs2_t[0:1, 0:RS]
                        nc.vector.tensor_copy(rden[:], rtmp[:])
                        prb = sc_tile()
                        nc.tensor.matmul(prb[:64, :RS], ones_1[:, :64], rden[:],
                                         start=True, stop=True)
                        rb = lnp.tile([64, RS], BF16, tag="dn_rb")
                        nc.scalar.copy(rb[:], prb[:64, :RS])
                        po = 64 * (h % 2)
                        nc.vector.tensor_tensor(attnT[po:po + 64, h // 2, :],
                                                pv_ps[i][:64, :RS], rb[:], OP.mult)

                # ---- bisection on DVE (overlaps dense on PE/ACT) ----
                spn_t = []
                for qi in range(QC if EN_SPARSE else 0):
                    Eb, lo_f, hi_f = Eb_t[qi], lo_t[qi], hi_t[qi]
                    c_lo = sm.tile([P, 1], F32, tag="se_cl")
                    c_hi = sm.tile([P, 1], F32, tag="se_ch")
                    nc.vector.memset(c_lo[:], float(S))
                    nc.vector.memset(c_hi[:], 1.0)
                    junk = sel2.tile([P, S], BF16, tag="jk", name="junk", bufs=2)
                    for it in range(N_BISECT):
                        t_f = sm.tile([P, 1], F32, tag="se_ti")
                        nc.vector.tensor_tensor(t_f[:].bitcast(I32),
                                                lo_f[:].bitcast(I32),
                                                hi_f[:].bitcast(I32), OP.add)
                        nc.vector.tensor_scalar(t_f[:].bitcast(I32),
                                                t_f[:].bitcast(I32), 1, None,
                                                op0=OP.logical_shift_right)
                        cnt = sm.tile([P, 1], F32, tag="se_cnt")
                        nc.vector.tensor_scalar(
                            junk[:], Eb[:], t_f[:], 0.0,
                            op0=OP.is_ge, op1=OP.add, accum_out=cnt[:])
                        ge = sm.tile([P, 1], I32, tag="se_ge")
                        lt = sm.tile([P, 1], I32, tag="se_lt")
                        nc.vector.tensor_scalar(ge[:], cnt[:], float(K_TOP), None,
                                                op0=OP.is_ge)
                        nc.vector.tensor_scalar(lt[:], cnt[:], float(K_TOP), None,
                                                op0=OP.is_lt)
                        nc.vector.copy_predicated(lo_f[:], ge[:], t_f[:])
                        nc.vector.copy_predicated(c_lo[:], ge[:], cnt[:])
                        nc.vector.copy_predicated(hi_f[:], lt[:], t_f[:])
                        nc.vector.copy_predicated(c_hi[:], lt[:], cnt[:])
                    # pick side with count closest to K_TOP
                    dlo = sm.tile([P, 1], F32, tag="se_dlo")
                    dhi = sm.tile([P, 1], F32, tag="se_dhi")
                    nc.vector.tensor_scalar(dlo[:], c_lo[:], float(K_TOP), None,
                                            op0=OP.subtract)
                    nc.vector.tensor_scalar(dhi[:], c_hi[:], -1.0, float(K_TOP),
                                            op0=OP.mult, op1=OP.add)
                    use_lo = sm.tile([P, 1], I32, tag="se_ul")
                    nc.vector.tensor_tensor(use_lo[:], dlo[:], dhi[:], OP.is_le)
                    t_f = sm.tile([P, 1], F32, tag="se_tf")
                    nc.vector.tensor_copy(t_f[:], hi_f[:])
                    nc.vector.copy_predicated(t_f[:], use_lo[:], lo_f[:])
                    ssel = sm.tile([P, 1], F32, tag="se_ss")
                    masked = sel2.tile([P, S], BF16, tag="jk", name="masked",
                                       bufs=2)
                    nc.vector.scalar_tensor_tensor(masked[:], Eb[:], t_f[:], Eb[:],
                                                   op0=OP.is_ge, op1=OP.mult,
                                                   accum_out=ssel[:])
                    rsel = sm.tile([P, 1], F32, tag="se_rs")
                    with nc.allow_low_precision(reason="sp renorm"):
                        nc.vector.reciprocal_approx_fast(rsel[:], ssel[:])
                    # x256 so the fp8 spT stays out of subnormal range
                    spn = sel2.tile([P, S], BF16, tag="spn", name="spn", bufs=4)
                    nc.vector.tensor_scalar(spn[:], masked[:], rsel[:], 256.0,
                                            op0=OP.mult, op1=OP.mult)
                    spn_t.append(spn)

                # ---- spT transposes (PE, after dense) + sparse PV ----
                spT = spt_pool.tile([P, SC, RS], F8, tag="spT", name="spT")
                for qi in range(QC if EN_SPARSE else 0):
                    spn = spn_t[qi]
                    for sc2 in range(SC // 4):
                        ptb = sc_tile_b()
                        for j in range(4):
                            scn = sc2 * 4 + j
                            nc.tensor.transpose(ptb[:, j * P:(j + 1) * P],
                                                spn[:, scn * P:(scn + 1) * P],
                                                identb[:])
                        nc.vector.tensor_copy(
                            spT[:, sc2 * 4:(sc2 + 1) * 4, qi * P:(qi + 1) * P],
                            ptb[:, :512].rearrange("p (a b) -> p a b", b=P))

                sp_sb = kv.tile([P, DK, RS], BF16, tag="sp_sb")
                if not EN_SPARSE:
                    nc.vector.memset(sp_sb[:], 0.0)
                for kk in range(DK if EN_SPARSE else 0):
                    pa = pspv.tile([P, 512], F32, tag="pspv", name="pa")
                    for scp in range(SC // 2):
                        nc.tensor.matmul(pa[:, :RS],
                                         Vs[:, 2 * scp:2 * scp + 2,
                                            kk * P:(kk + 1) * P],
                                         spT[:, 2 * scp:2 * scp + 2, :],
                                         start=(scp == 0), stop=(scp == SC // 2 - 1),
                                         perf_mode=DR)
                    nc.scalar.activation(sp_sb[:, kk, :], pa[:, :RS], AF.Copy,
                                         scale=1.0 / 256.0)

                # ---- out proj + gating -> u1; LN1 -> y ----
                u1 = sel.tile([P, DK, RS], F32R, tag="E", name="u1")
                for kk in range(DK):
                    ow_s = wf.tile([P, DK, P], BF16, tag="ow")
                    nc.gpsimd.dma_start(ow_s[:], ow_d.ap()[l, :, :, kk * P:(kk + 1) * P])
                    pd = hold_tile()
                    for kk2 in range(DK):
                        nc.tensor.matmul(pd[:, :RS], ow_s[:, kk2, :],
                                         attnT[:, kk2, :],
                                         start=(kk2 == 0), stop=(kk2 == DK - 1))
                    nc.vector.scalar_tensor_tensor(
                        u1[:, kk, :], pd[:, :RS], g_all[:, l:l + 1], xo[:, kk, :],
                        op0=OP.mult, op1=OP.add)
                    nc.vector.scalar_tensor_tensor(
                        u1[:, kk, :], sp_sb[:, kk, :], gm_all[:, l:l + 1],
                        u1[:, kk, :], op0=OP.mult, op1=OP.add)
                y = st.tile([P, DK, RS], F32R, tag="y")
                layernorm(u1, l, 0, y)

                # ---- FFN (bf16) ----
                yB = kv.tile([P, DK, RS], BF16, tag="yB")
                nc.vector.tensor_copy(yB[:], y[:])
                hT = spt_pool.tile([P, SC, RS], BF16, tag="hT", name="hT")
                if not EN_FFN:
                    nc.vector.memset(hT[:], 0.0)
                for fg in range(4 if EN_FFN else 0):
                    f1_s = wf.tile([P, DK, 4 * P], BF16, tag="f1")
                    nc.gpsimd.dma_start(f1_s[:], f1_d.ap()[l, :, :, fg * 512:(fg + 1) * 512])
                    for j in range(4):
                        oc = fg * 4 + j
                        ph = sc_tile()
                        for kk in range(DK):
                            nc.tensor.matmul(ph[:, :RS],
                                             f1_s[:, kk, j * P:(j + 1) * P],
                                             yB[:, kk, :],
                                             start=(kk == 0), stop=(kk == DK - 1))
                        nc.scalar.activation(hT[:, oc, :], ph[:, :RS], AF.Relu)
                u2 = sel.tile([P, DK, RS], F32R, tag="E", name="u2")
                for kk in range(DK):
                    f2_s = wf2.tile([P, FFC, P], BF16, tag="f2", bufs=2)
                    nc.gpsimd.dma_start(f2_s[:], f2_d.ap()[l, :, :, kk * P:(kk + 1) * P])
                    pf = hold_tile()
                    for oc in range(FFC):
                        nc.tensor.matmul(pf[:, :RS],
                                         f2_s[:, oc, :],
                                         hT[:, oc, :],
                                         start=(oc == 0), stop=(oc == FFC - 1))
                    nc.vector.tensor_tensor(u2[:, kk, :], pf[:, :RS], y[:, kk, :],
                                            OP.add)
                layernorm(u2, l, 1, xo)
                if dump_x:
                    nc.sync.dma_start(dbg_d.ap()[l + 1], xo[:].bitcast(F32))
                if l < L_RUN - 1:
                    xo8_cur = kv.tile([P, DK, RS], F8, tag="xo8", name="xo8n")
                    nc.vector.tensor_copy(xo8_cur[:], xo[:])
                    allgather_x8(xo8_cur)

            if os.environ.get("K_PRINT"):
                import contextlib
                with open("/tmp/prog.txt", "w") as f:
                    with contextlib.redirect_stdout(f):
                        nc.print_concise(deps=True)
            ctx2.close()
            # ---- final AllGather in bf16 for the logit matmul ----
            fin2 = ctx.enter_context(tc.tile_pool(name="fin2", bufs=2))
            xTb = fin2.tile([P, DK, S], BF16, tag="xTb", bufs=1)
            xoBf = fin2.tile([P, DK, RS], BF16, tag="xoBf", bufs=1)
            nc.vector.tensor_copy(xoBf[:], xo[:])
            ag_inb = dram.tile([P, DK, RS], BF16)
            ag_outb = dram.tile([4, P, DK, RS], BF16)
            nc.sync.dma_start(ag_inb[:], xoBf[:])
            nc.gpsimd.collective_compute(
                "AllGather", OP.bypass,
                replica_groups=[[0, 1, 2, 3], [4, 5, 6, 7]],
                ins=[ag_inb[:].opt()], outs=[ag_outb[:].opt()])
            for rr in range(4):
                nc.sync.dma_start(xTb[:, :, rr * RS:(rr + 1) * RS], ag_outb[rr])

            # ================= final projection =================
            NVC = (VSL + P - 1) // P
            for vc in range(NVC):
                grp, off = vc // 4, vc % 4
                if off == 0:
                    ftile = fin2.tile([P, DK, 4 * P], BF16, tag="fin")
                    w = min(4 * P, VSL - grp * 4 * P)
                    nc.gpsimd.dma_start(ftile[:, :, :w],
                                      fin_d.ap()[:, :, grp * 4 * P:grp * 4 * P + w])
                vw = min(P, VSL - vc * P)
                for rc in range(2):
                    pl = sc_tile()
                    for half in range(2):
                        for kk in range(DK):
                            nc.tensor.matmul(
                                pl[:vw, half * 512:(half + 1) * 512],
                                ftile[:, kk, off * P:off * P + vw],
                                xTb[:, kk,
                                    rc * 1024 + half * 512:rc * 1024 + (half + 1) * 512],
                                start=(kk == 0), stop=(kk == DK - 1))
                    lo_s = fin2.tile([P, 1024], BF16, tag="lo")
                    if (vc + rc) % 2 == 0:
                        nc.scalar.copy(lo_s[:vw, :], pl[:vw, :])
                    else:
                        nc.vector.tensor_copy(lo_s[:vw, :], pl[:vw, :])
                    nc.sync.dma_start(
                        out_d.ap()[vc * P:vc * P + vw, rc * 1024:(rc + 1) * 1024],
                        lo_s[:vw, :])

    nc.compile()
    return nc


def _prep_inputs(inputs):
    f32 = np.float32
    bf = ml_dtypes.bfloat16
    f8 = ml_dtypes.float8_e4m3
    src = np.asarray(inputs["src"]).astype(np.int64)
    emb = np.ascontiguousarray(np.asarray(inputs["emb"], f32))
    pe = np.asarray(inputs["pe"], f32)
    lam = np.asarray(inputs["lam"], f32)
    for nm in ("in_b", "out_b", "qp_b", "kp_b", "vp_b", "ff1_b", "ff2_b", "fin_b"):
        assert not np.any(np.asarray(inputs[nm])), f"nonzero bias {nm} unsupported"
    in_w = np.asarray(inputs["in_w"], f32)
    out_w = np.asarray(inputs["out_w"], f32)
    qp_w = np.asarray(inputs["qp_w"], f32)
    kp_w = np.asarray(inputs["kp_w"], f32)
    vp_w = np.asarray(inputs["vp_w"], f32)
    ff1_w = np.asarray(inputs["ff1_w"], f32)
    ff2_w = np.asarray(inputs["ff2_w"], f32)
    ln1_s = np.asarray(inputs["ln1_s"], f32)
    ln1_b = np.asarray(inputs["ln1_b"], f32)
    ln2_s = np.asarray(inputs["ln2_s"], f32)
    ln2_b = np.asarray(inputs["ln2_b"], f32)
    fin_w = np.asarray(inputs["fin_w"], f32)

    def to_pdk(w):  # [L, D, C] -> [L, P, DK, C]
        Lx, Dx, Cx = w.shape
        return np.ascontiguousarray(
            w.reshape(Lx, DK, P, Cx).transpose(0, 2, 1, 3))

    def to_f8(w):
        return np.clip(w * WSCALE, -240.0, 240.0).astype(f8)

    wl = to_f8(to_pdk(np.concatenate([
        in_w[:, 512:1024, :].transpose(0, 2, 1),
        kp_w.transpose(0, 2, 1),
        in_w[:, 0:512, :].transpose(0, 2, 1),
        qp_w.transpose(0, 2, 1),
    ], axis=2)))
    wr = to_f8(to_pdk(np.concatenate([
        in_w[:, 1024:1536, :].transpose(0, 2, 1),
        vp_w.transpose(0, 2, 1),
    ], axis=2)))
    ow = to_pdk(out_w.transpose(0, 2, 1)).astype(bf)
    f1 = to_pdk(ff1_w.transpose(0, 2, 1)).astype(bf)
    f2 = np.ascontiguousarray(
        ff2_w.transpose(0, 2, 1).reshape(L, FFC, P, D).transpose(0, 2, 1, 3)).astype(bf)
    lns = np.ascontiguousarray(
        np.stack([ln1_s, ln2_s], 1).reshape(L, 2, DK, P).transpose(0, 1, 3, 2))
    lnb = np.ascontiguousarray(
        np.stack([ln1_b, ln2_b], 1).reshape(L, 2, DK, P).transpose(0, 1, 3, 2))
    peT = np.ascontiguousarray(pe.T.reshape(DK, P, S).transpose(1, 0, 2))
    finT = np.ascontiguousarray(fin_w.T.reshape(DK, P, V).transpose(1, 0, 2))

    in_maps = []
    for c in range(NCORE):
        b, r = c // 4, c % 4
        in_maps.append({
            "emb": emb,
            "idxo": _wrap_idx(src[b, r * RS:(r + 1) * RS]),
            "peTo": np.ascontiguousarray(peT[:, :, r * RS:(r + 1) * RS]),
            "lam": lam.reshape(1, L).astype(f32),
            "wl": wl, "wr": wr, "ow": ow, "f1": f1, "f2": f2,
            "lns": lns, "lnb": lnb,
            "fin": np.ascontiguousarray(
                finT[:, :, r * VSL:(r + 1) * VSL]).astype(bf),
        })
    return in_maps


def kernel(**inputs):
    dump_x = bool(int(os.environ.get("KERNEL_DUMP_X", "0")))
    key = ("nc", dump_x)
    if key not in _CACHE:
        _CACHE[key] = build_nc(dump_x)
    nc = _CACHE[key]
    in_maps = _prep_inputs(inputs)
    trace = bool(int(os.environ.get("KERNEL_TRACE", "0")))
    res = run_bass_kernel_spmd(nc, in_maps, core_ids=list(range(NCORE)),
                               trace=trace)
    if trace:
        _CACHE["last_res"] = res
    out = np.zeros((B, S, V), np.float32)
    for c in range(NCORE):
        b, r = c // 4, c % 4
        out[b, :, r * VSL:(r + 1) * VSL] = res.results[c]["out"].T.astype(np.float32)
    if dump_x:
        _CACHE["dbg"] = [res.results[c].get("dbg") for c in range(NCORE)]
    return out


# revision 23
# speedup vs baseline: 1.7956x; 1.0626x over previous
# Trainium2 Bass kernel for nn_EnhancedEURLTransformer_87694642249910
# Sharding: 8 cores = 2 (batch) x 4 (sequence rows). Per-layer AllGather of x
# within each 4-core group (fp8 payload; final AG bf16 for the logit matmul).
# Activations transposed [D on partitions, rows free]. fp8e4 DoubleRow matmuls
# for all D-contraction projections and both PV contractions; FFN / out-proj /
# final projection stay bf16. Sparse top-K threshold found by a warm-started
# bisection seeded from per-row log-normal stats of exp(qk/8).
import os
import sys

sys.path.insert(0, "/opt/trn_rl_repo")

import math
import numpy as np
import ml_dtypes

import concourse.bass as bass
import concourse.mybir as mybir
import concourse.tile as tile
from concourse import bacc
from concourse.bass_utils import run_bass_kernel_spmd
from concourse.masks import make_identity

B, S, D, H, R, L, V, FF = 2, 2048, 512, 8, 64, 6, 32000, 2048
HD = D // H          # 64
K_TOP = 409
LN_EPS = 1e-5

F32 = mybir.dt.float32
F32R = mybir.dt.float32r
BF16 = mybir.dt.bfloat16
F8 = mybir.dt.float8e4
I16 = mybir.dt.int16
I32 = mybir.dt.int32
AF = mybir.ActivationFunctionType
OP = mybir.AluOpType
AX = mybir.AxisListType
DR = mybir.MatmulPerfMode.DoubleRow

P = 128
DK = D // P          # 4 d-chunks
SC = S // P          # 16 seq-chunks
NCORE = 8
RS = S // 4          # 512 rows per core
QC = RS // P         # 4 own-row chunks
VSL = V // 4         # 8000 vocab cols per core
FFC = FF // P        # 16
SQRT_D = math.sqrt(D)
WSCALE = 64.0        # fp8 projection-weight prescale (avoids subnormals)
LN_N = math.log(float(S))
Z_LO, Z_HI = 0.72, 0.97

N_BISECT = int(os.environ.get("K_BISECT", "5"))
L_RUN = int(os.environ.get("K_LAYERS", str(L)))
EN_SPARSE = os.environ.get("K_SPARSE", "1") == "1"
EN_DENSE = os.environ.get("K_DENSE", "1") == "1"
EN_FFN = os.environ.get("K_FFN", "1") == "1"

_CACHE = {}


def _wrap_idx(idx):
    # dma_gather index wrapping: token i -> partition i%16, col i//16
    # tile must be [128, n//16]; only partitions 0..15 are read
    n = idx.shape[0]
    return np.ascontiguousarray(
        np.tile(idx.reshape(n // 16, 16).T.astype(np.int16), (8, 1)))


def build_nc(dump_x=False):
    nc = bacc.Bacc("TRN2", target_bir_lowering=False, debug=False, num_devices=NCORE)

    emb_d = nc.dram_tensor("emb", [V, D], F32, kind="ExternalInput")
    idxo_d = nc.dram_tensor("idxo", [128, RS // 16], I16, kind="ExternalInput")
    peTo_d = nc.dram_tensor("peTo", [P, DK, RS], F32, kind="ExternalInput")
    lam_d = nc.dram_tensor("lam", [1, L], F32R, kind="ExternalInput")
    wl_d = nc.dram_tensor("wl", [L, P, DK, 1152], F8, kind="ExternalInput")
    wr_d = nc.dram_tensor("wr", [L, P, DK, 1024], F8, kind="ExternalInput")
    ow_d = nc.dram_tensor("ow", [L, P, DK, D], BF16, kind="ExternalInput")
    f1_d = nc.dram_tensor("f1", [L, P, DK, FF], BF16, kind="ExternalInput")
    f2_d = nc.dram_tensor("f2", [L, P, FFC, D], BF16, kind="ExternalInput")
    lns_d = nc.dram_tensor("lns", [L, 2, P, DK], F32, kind="ExternalInput")
    lnb_d = nc.dram_tensor("lnb", [L, 2, P, DK], F32, kind="ExternalInput")
    fin_d = nc.dram_tensor("fin", [P, DK, VSL], BF16, kind="ExternalInput")
    out_d = nc.dram_tensor("out", [VSL, S], BF16, kind="ExternalOutput")
    if dump_x:
        dbg_d = nc.dram_tensor("dbg", [L + 1, P, DK, RS], F32, kind="ExternalOutput")

    from contextlib import ExitStack
    with tile.TileContext(nc) as tc, ExitStack() as ctx:
        ep = ctx.enter_context
        st = ep(tc.tile_pool(name="state", bufs=1))
        sm = ep(tc.tile_pool(name="small", bufs=2))
        psc = ep(tc.tile_pool(name="psc", bufs=2, space="PSUM"))
        pspv = ep(tc.tile_pool(name="pspv", bufs=1, space="PSUM"))
        phold = ep(tc.tile_pool(name="phold", bufs=2, space="PSUM"))
        dram = ep(tc.tile_pool(name="dram", bufs=1, space="DRAM"))
        ctx2 = ctx.enter_context(ExitStack())
        ep2 = ctx2.enter_context
        wp = ep2(tc.tile_pool(name="wproj", bufs=1))
        wf = ep2(tc.tile_pool(name="wffn", bufs=2))
        wf2 = ep2(tc.tile_pool(name="wf2", bufs=1))
        kv = ep2(tc.tile_pool(name="kv", bufs=1))
        sel = ep2(tc.tile_pool(name="sel", bufs=1))
        sel2 = ep2(tc.tile_pool(name="sel2", bufs=1))
        spt_pool = ep2(tc.tile_pool(name="spTp", bufs=1))
        expp = ep2(tc.tile_pool(name="expp", bufs=2))
        lnp = ep2(tc.tile_pool(name="lnp", bufs=1))
        if True:
            def sc_tile():
                return psc.tile([P, 1024], F32, tag="sc", name="sc")

            def sc_tile_b():
                return psc.tile([P, 1024], BF16, tag="sc", name="scb")

            def hold_tile():
                return phold.tile([P, 512], F32, tag="hold", name="hold")

            # ------------- persistent state -------------
            xT8 = st.tile([P, DK, S], F8)           # gathered x (all rows, fp8)
            xo = st.tile([P, DK, RS], F32R)         # own rows, residual spine
            ident = st.tile([P, P], F32)
            identb = st.tile([P, P], BF16)
            ones_f32r = st.tile([P, 1], F32R)
            ones_1 = st.tile([1, P], F32R)
            g_all = st.tile([P, L], F32)
            gm_all = st.tile([P, L], F32)
            make_identity(nc, ident)
            make_identity(nc, identb)
            ones_tmp = sm.tile([P, 1], F32, tag="otmp")
            nc.vector.memset(ones_tmp[:], 1.0)
            nc.vector.tensor_copy(ones_f32r[:], ones_tmp[:])
            ones_tmp2 = sm.tile([1, P], F32, tag="otmp2")
            nc.vector.memset(ones_tmp2[:], 1.0)
            nc.vector.tensor_copy(ones_1[:], ones_tmp2[:])

            eps_t = st.tile([1, 1], F32)
            nc.vector.memset(eps_t[:], LN_EPS)
            eps_p = st.tile([P, 1], F32)
            nc.vector.memset(eps_p[:], 1e-6)
            lam_s = sm.tile([1, L], F32R, tag="lam")
            nc.sync.dma_start(lam_s[:], lam_d.ap())
            g_row = sm.tile([1, L], F32R, tag="lam")
            nc.scalar.activation(g_row[:], lam_s[:], AF.Sigmoid)
            pg = sc_tile()
            nc.tensor.matmul(pg[:, :L], ones_1[:], g_row[:], start=True, stop=True)
            nc.vector.tensor_copy(g_all[:], pg[:, :L])
            nc.vector.tensor_scalar(gm_all[:], g_all[:], -1.0, 1.0,
                                    op0=OP.mult, op1=OP.add)  # 1-g

            ag_in8h = [dram.tile([P, 2, RS], F8, name=f"agi{h}")
                       for h in range(2)]
            ag_out8h = [dram.tile([4, P, 2, RS], F8, name=f"ago{h}")
                        for h in range(2)]

            # ---------- embedding: own rows ----------
            idx_s = sm.tile([128, RS // 16], I16, tag="idx")
            nc.sync.dma_start(idx_s[:], idxo_d.ap())
            gath = sel.tile([P, QC, D], F32, tag="E", name="gath")
            nc.gpsimd.dma_gather(gath[:], emb_d.ap(), idx_s[:], RS, RS, D)
            for kk in range(DK):
                pt = sc_tile()
                for c in range(QC):
                    nc.tensor.transpose(pt[:, c * P:(c + 1) * P],
                                        gath[:, c, kk * P:(kk + 1) * P], ident[:])
                nc.vector.tensor_scalar_mul(xo[:, kk, :].bitcast(F32), pt[:, :RS],
                                            SQRT_D)
            nc.gpsimd.dma_start(xo[:].bitcast(F32), peTo_d.ap(),
                                accum_op=OP.add)

            def allgather_x8(xo8_src):
                # split by kk-halves: consumers contract pairs {0,1} first,
                # so half 0 unblocks the next layer's projections early
                for h in range(2):
                    nc.sync.dma_start(ag_in8h[h][:],
                                      xo8_src[:, 2 * h:2 * h + 2, :])
                    nc.gpsimd.collective_compute(
                        "AllGather", OP.bypass,
                        replica_groups=[[0, 1, 2, 3], [4, 5, 6, 7]],
                        ins=[ag_in8h[h][:].opt()], outs=[ag_out8h[h][:].opt()])
                for h in range(2):
                    for rr in range(4):
                        nc.sync.dma_start(
                            xT8[:, 2 * h:2 * h + 2, rr * RS:(rr + 1) * RS],
                            ag_out8h[h][rr])

            if dump_x:
                nc.sync.dma_start(dbg_d.ap()[0], xo[:].bitcast(F32))
            xo8_cur = kv.tile([P, DK, RS], F8, tag="xo8", name="xo8e")
            nc.vector.tensor_copy(xo8_cur[:], xo[:])
            allgather_x8(xo8_cur)

            rs_t = lnp.tile([128, 3 * RS], F32, name="rowscratch")
            rs2_t = lnp.tile([1, 3 * RS], F32R, name="rowscratch2")
            # ---------- layernorm: dst = LN(u) ----------
            def layernorm(u, l, which, dst):
                usq_t = []
                for kk in range(DK):
                    usq = lnp.tile([P, RS], F32R, tag="usq", name="usq")
                    nc.vector.tensor_tensor(usq, u[:, kk, :], u[:, kk, :], OP.mult)
                    usq_t.append(usq)
                psum_s = hold_tile()
                for kk in range(DK):
                    nc.tensor.matmul(psum_s[:1, :RS], ones_f32r[:], u[:, kk, :],
                                     start=(kk == 0), stop=(kk == DK - 1))
                mean = rs_t[0:1, 0:RS]
                nc.vector.tensor_scalar_mul(mean[:], psum_s[:1, :RS], 1.0 / D)
                pssq = hold_tile()
                for kk in range(DK):
                    nc.tensor.matmul(pssq[:1, :RS], ones_f32r[:], usq_t[kk][:],
                                     start=(kk == 0), stop=(kk == DK - 1))
                msq = rs_t[64:65, 0:RS]
                nc.vector.tensor_tensor(msq[:], mean[:], mean[:], OP.mult)
                var = rs_t[32:33, 0:RS]
                nc.vector.scalar_tensor_tensor(var[:], pssq[:1, :RS], 1.0 / D,
                                               msq[:], op0=OP.mult,
                                               op1=OP.subtract)
                sd = rs_t[0:1, RS:2 * RS]
                nc.scalar.activation(sd[:], var[:], AF.Sqrt, bias=eps_t[:])
                rtmp = rs_t[0:1, 2 * RS:3 * RS]
                with nc.allow_low_precision(reason="f32r istd"):
                    nc.vector.reciprocal_approx_fast(rtmp[:], sd[:])
                istd = rs2_t[0:1, 0:RS]
                nc.vector.tensor_copy(istd[:], rtmp[:])
                nistd = rs2_t[0:1, RS:2 * RS]
                nc.vector.tensor_tensor(nistd[:], mean[:].bitcast(F32R), istd[:],
                                        OP.mult)
                nc.vector.tensor_scalar_mul(nistd[:], nistd[:], -1.0)
                pA = hold_tile()
                nc.tensor.matmul(pA[:, :RS], ones_1[:], istd[:], start=True, stop=True)
                pB = hold_tile()
                nc.tensor.matmul(pB[:, :RS], ones_1[:], nistd[:], start=True, stop=True)
                scl = sm.tile([P, DK], F32, tag="ln_sc")
                bcl = sm.tile([P, DK], F32, tag="ln_bc")
                nc.sync.dma_start(scl[:], lns_d.ap()[l, which])
                nc.sync.dma_start(bcl[:], lnb_d.ap()[l, which])
                for kk in range(DK):
                    t0 = dst[:, kk, :]
                    nc.vector.tensor_tensor(t0, u[:, kk, :],
                                            pA[:, :RS].bitcast(F32R), OP.mult)
                    nc.vector.tensor_tensor(t0, t0, pB[:, :RS].bitcast(F32R), OP.add)
                    nc.vector.tensor_scalar(
                        t0, t0, scl[:, kk:kk + 1], bcl[:, kk:kk + 1],
                        op0=OP.mult, op1=OP.add)

            # ================= layers =================
            for l in range(L_RUN):
                wl_s = wp.tile([P, DK, 1152], F8, tag="wl")
                wr_s = wp.tile([P, DK, 1024], F8, tag="wr")
                nc.gpsimd.dma_start(wl_s[:], wl_d.ap()[l])
                nc.gpsimd.dma_start(wr_s[:], wr_d.ap()[l])
                xo8 = xo8_cur

                # ---- q^T, Qs^T from own rows (fp8 DoubleRow pairs) ----
                qT = kv.tile([P, DK, RS], F8, tag="qT")
                QsT = kv.tile([64, RS], F8, tag="QsT")
                for oc in range(DK):
                    pq = sc_tile()
                    for kk2 in (0, 2):
                        nc.tensor.matmul(pq[:, :RS],
                                         wl_s[:, kk2:kk2 + 2,
                                              576 + oc * P:576 + (oc + 1) * P],
                                         xo8[:, kk2:kk2 + 2, :],
                                         start=(kk2 == 0), stop=(kk2 == 2),
                                         perf_mode=DR)
                    nc.scalar.activation(qT[:, oc, :], pq[:, :RS], AF.Copy,
                                         scale=1.0 / WSCALE)
                pq = sc_tile()
                for kk2 in (0, 2):
                    nc.tensor.matmul(pq[:64, :RS], wl_s[:, kk2:kk2 + 2, 1088:1152],
                                     xo8[:, kk2:kk2 + 2, :],
                                     start=(kk2 == 0), stop=(kk2 == 2),
                                     perf_mode=DR)
                nc.scalar.activation(QsT[:], pq[:64, :RS], AF.Copy,
                                     scale=1.0 / WSCALE)

                # ---- k^T, Ks^T (full seq, fp8 DoubleRow) ----
                kT = kv.tile([P, DK, S], F8, tag="kT")
                KsT = kv.tile([64, S], F8, tag="KsT")
                for oc in range(DK):
                    for fc in range(S // 1024):
                        pk = sc_tile()
                        for hh in range(2):
                            for kk2 in (0, 2):
                                nc.tensor.matmul(
                                    pk[:, hh * 512:(hh + 1) * 512],
                                    wl_s[:, kk2:kk2 + 2, oc * P:(oc + 1) * P],
                                    xT8[:, kk2:kk2 + 2,
                                        fc * 1024 + hh * 512:fc * 1024 + (hh + 1) * 512],
                                    start=(kk2 == 0), stop=(kk2 == 2),
                                    perf_mode=DR)
                        if oc % 2 == 0:
                            nc.vector.tensor_scalar_mul(
                                kT[:, oc, fc * 1024:(fc + 1) * 1024], pk[:],
                                1.0 / WSCALE)
                        else:
                            nc.scalar.activation(
                                kT[:, oc, fc * 1024:(fc + 1) * 1024], pk[:],
                                AF.Copy, scale=1.0 / WSCALE)
                for fc in range(S // 1024):
                    pk = sc_tile()
                    for hh in range(2):
                        for kk2 in (0, 2):
                            nc.tensor.matmul(
                                pk[:64, hh * 512:(hh + 1) * 512],
                                wl_s[:, kk2:kk2 + 2, 512:576],
                                xT8[:, kk2:kk2 + 2,
                                    fc * 1024 + hh * 512:fc * 1024 + (hh + 1) * 512],
                                start=(kk2 == 0), stop=(kk2 == 2),
                                perf_mode=DR)
                    nc.scalar.activation(KsT[:, fc * 1024:(fc + 1) * 1024],
                                         pk[:64, :], AF.Copy, scale=1.0 / WSCALE)

                # ---- sparse E = exp(qk/8) + warm-start stats ----
                # per qi: Eb [P,S] bf16; accums S1 = sum E, S2 = sum E^2.
                # Stats batched [P,4] across qi so the ACT table loads once
                # per function instead of thrashing Exp/Ln/Sqrt per chunk.
                if EN_SPARSE:
                    Eb_t = []
                    s14 = sm.tile([P, 4], F32, tag="se_s14")
                    s24 = sm.tile([P, 4], F32, tag="se_s24")
                for qi in range(QC if EN_SPARSE else 0):
                    Eb = sel2.tile([P, S], BF16, tag="Eb", name="Eb", bufs=4)
                    a1 = sm.tile([P, 2], F32, tag="se_a1")
                    a2 = sm.tile([P, 2], F32, tag="se_a2")
                    for half in range(2):
                        pq_ = sc_tile()
                        nc.tensor.matmul(pq_[:, :512],
                                         QsT[:, qi * P:(qi + 1) * P],
                                         KsT[:, half * 1024:half * 1024 + 512],
                                         start=True, stop=True)
                        nc.tensor.matmul(pq_[:, 512:],
                                         QsT[:, qi * P:(qi + 1) * P],
                                         KsT[:, half * 1024 + 512:(half + 1) * 1024],
                                         start=True, stop=True)
                        nc.scalar.activation(Eb[:, half * 1024:(half + 1) * 1024],
                                             pq_[:], AF.Exp, scale=0.125,
                                             accum_out=a1[:, half:half + 1])
                        junk2 = sel2.tile([P, S], BF16, tag="jk", name="jk2",
                                          bufs=2)
                        nc.scalar.activation(junk2[:, :1024], pq_[:], AF.Exp,
                                             scale=0.25,
                                             accum_out=a2[:, half:half + 1])
                    nc.vector.tensor_tensor(s14[:, qi:qi + 1], a1[:, 0:1],
                                            a1[:, 1:2], OP.add)
                    nc.vector.tensor_tensor(s24[:, qi:qi + 1], a2[:, 0:1],
                                            a2[:, 1:2], OP.add)
                    Eb_t.append(Eb)
                if EN_SPARSE:
                    # log-normal stats -> bisect brackets lo4/hi4 [P,4]
                    L14 = sm.tile([P, 4], F32, tag="se_l14")
                    L24 = sm.tile([P, 4], F32, tag="se_l24")
                    nc.scalar.activation(L14[:], s14[:], AF.Ln)
                    nc.scalar.activation(L24[:], s24[:], AF.Ln)
                    sig2 = sm.tile([P, 4], F32, tag="se_sg2")
                    nc.vector.scalar_tensor_tensor(sig2[:], L14[:], -2.0, L24[:],
                                                   op0=OP.mult, op1=OP.add)
                    nc.vector.tensor_scalar(sig2[:], sig2[:], 1.0, LN_N,
                                            op0=OP.mult, op1=OP.add)
                    sig = sm.tile([P, 4], F32, tag="se_sg")
                    nc.scalar.activation(sig[:], sig2[:], AF.Sqrt, bias=eps_p[:])
                    mu = sm.tile([P, 4], F32, tag="se_mu")
                    nc.vector.tensor_scalar(mu[:], sig2[:], -0.5, -LN_N,
                                            op0=OP.mult, op1=OP.add)
                    nc.vector.tensor_tensor(mu[:], mu[:], L14[:], OP.add)
                    ulo = sm.tile([P, 4], F32, tag="se_ulo")
                    uhi = sm.tile([P, 4], F32, tag="se_uhi")
                    nc.vector.scalar_tensor_tensor(ulo[:], sig[:], Z_LO, mu[:],
                                                   op0=OP.mult, op1=OP.add)
                    nc.vector.scalar_tensor_tensor(uhi[:], sig[:], Z_HI, mu[:],
                                                   op0=OP.mult, op1=OP.add)
                    lo4 = sm.tile([P, 4], F32, tag="se_lo4", bufs=2)
                    hi4 = sm.tile([P, 4], F32, tag="se_hi4", bufs=2)
                    nc.scalar.activation(lo4[:], ulo[:], AF.Exp)
                    nc.scalar.activation(hi4[:], uhi[:], AF.Exp)

                # ---- v520 (ones col per head), Vs (fp8 DoubleRow) ----
                v520 = kv.tile([P, SC, 8 * 80], F8, tag="v520")
                Vs = kv.tile([P, SC, D], F8, tag="Vs")
                if l == 0:
                    nc.vector.memset(
                        v520[:].rearrange("p s (h c) -> p s h c", c=80)[:, :, :, 64:65],
                        1.0)
                for scn in range(SC):
                    pv_ = sc_tile()
                    for kk2 in (0, 2):
                        nc.tensor.matmul(pv_[:, :512],
                                         xT8[:, kk2:kk2 + 2, scn * P:(scn + 1) * P],
                                         wr_s[:, kk2:kk2 + 2, 0:512],
                                         start=(kk2 == 0), stop=(kk2 == 2),
                                         perf_mode=DR)
                    nc.vector.tensor_scalar_mul(
                        v520[:, scn, :].rearrange("p (h c) -> p h c", c=80)[:, :, :64],
                        pv_[:, :512].rearrange("p (h c) -> p h c", c=64),
                        1.0 / WSCALE)
                    pv2 = sc_tile()
                    for kk2 in (0, 2):
                        nc.tensor.matmul(pv2[:, :512],
                                         xT8[:, kk2:kk2 + 2, scn * P:(scn + 1) * P],
                                         wr_s[:, kk2:kk2 + 2, 512:1024],
                                         start=(kk2 == 0), stop=(kk2 == 2),
                                         perf_mode=DR)
                    nc.scalar.activation(Vs[:, scn, :], pv2[:, :512], AF.Copy,
                                         scale=1.0 / WSCALE)

                # ---- dense attention (scores fp8 K=64; PV fp8 DoubleRow) ----
                attnT = kv.tile([P, DK, RS], BF16, tag="attnT")
                if not EN_DENSE:
                    nc.vector.memset(attnT[:], 0.0)
                for hp in range(4 if EN_DENSE else 0):
                    pv_ps = [hold_tile(), hold_tile()]
                    for scp in range(SC // 2):
                        eTp = expp.tile([P, 2, 1024], F8, tag="eT")
                        for j in range(2):
                            scn = 2 * scp + j
                            psum_sc = sc_tile()
                            for i, h in enumerate((2 * hp, 2 * hp + 1)):
                                po = 64 * (h % 2)
                                nc.tensor.matmul(
                                    psum_sc[:, i * 512:(i + 1) * 512],
                                    kT[po:po + 64, h // 2, scn * P:(scn + 1) * P],
                                    qT[po:po + 64, h // 2, :],
                                    start=True, stop=True)
                            nc.scalar.activation(eTp[:, j, :], psum_sc[:], AF.Exp,
                                                 scale=0.125)
                        for i, h in enumerate((2 * hp, 2 * hp + 1)):
                            nc.tensor.matmul(
                                pv_ps[i][:65, :RS],
                                v520[:, 2 * scp:2 * scp + 2, h * 80:h * 80 + 65],
                                eTp[:, :, i * 512:(i + 1) * 512],
                                start=(scp == 0), stop=(scp == SC // 2 - 1),
                                perf_mode=DR)
                    for i, h in enumerate((2 * hp, 2 * hp + 1)):
                        den = rs_t[0:1, RS:2 * RS]
                        nc.scalar.copy(den[:], pv_ps[i][64:65, :RS])
                        rtmp = rs_t[0:1, 2 * RS:3 * RS]
                        with nc.allow_low_precision(reason="f32r rden"):
                            nc.vector.reciprocal_approx_fast(rtmp[:], den[:])
                        rden = rs2_t[0:1, 0:RS]
                        nc.vector.tensor_copy(rden[:], rtmp[:])
                        prb = sc_tile()
                        nc.tensor.matmul(prb[:64, :RS], ones_1[:, :64], rden[:],
                                         start=True, stop=True)
                        rb = lnp.tile([64, RS], BF16, tag="dn_rb")
                        nc.scalar.copy(rb[:], prb[:64, :RS])
                        po = 64 * (h % 2)
                        nc.vector.tensor_tensor(attnT[po:po + 64, h // 2, :],
                                                pv_ps[i][:64, :RS], rb[:], OP.mult)

                # ---- bisection on DVE (overlaps dense on PE/ACT) ----
                spn_t = []
                for qi in range(QC if EN_SPARSE else 0):
                    Eb = Eb_t[qi]
                    lo_f = lo4[:, qi:qi + 1]
                    hi_f = hi4[:, qi:qi + 1]
                    c_lo = sm.tile([P, 1], F32, tag="se_cl")
                    c_hi = sm.tile([P, 1], F32, tag="se_ch")
                    nc.vector.memset(c_lo[:], float(S))
                    nc.vector.memset(c_hi[:], 1.0)
                    junk = sel2.tile([P, S], BF16, tag="jk", name="junk", bufs=2)
                    for it in range(N_BISECT):
                        t_f = sm.tile([P, 1], F32, tag="se_ti")
                        nc.vector.tensor_tensor(t_f[:].bitcast(I32),
                                                lo_f[:].bitcast(I32),
                                                hi_f[:].bitcast(I32), OP.add)
                        nc.vector.tensor_scalar(t_f[:].bitcast(I32),
                                                t_f[:].bitcast(I32), 1, None,
                                                op0=OP.logical_shift_right)
                        cnt = sm.tile([P, 1], F32, tag="se_cnt")
                        nc.vector.tensor_scalar(
                            junk[:], Eb[:], t_f[:], 0.0,
                            op0=OP.is_ge, op1=OP.add, accum_out=cnt[:])
                        ge = sm.tile([P, 1], I32, tag="se_ge")
                        lt = sm.tile([P, 1], I32, tag="se_lt")
                        nc.vector.tensor_scalar(ge[:], cnt[:], float(K_TOP), None,
                                                op0=OP.is_ge)
                        nc.vector.tensor_scalar(lt[:], cnt[:], float(K_TOP), None,
                                                op0=OP.is_lt)
                        nc.vector.copy_predicated(lo_f[:], ge[:], t_f[:])
                        nc.vector.copy_predicated(c_lo[:], ge[:], cnt[:])
                        nc.vector.copy_predicated(hi_f[:], lt[:], t_f[:])
                        nc.vector.copy_predicated(c_hi[:], lt[:], cnt[:])
                    # pick side with count closest to K_TOP
                    dlo = sm.tile([P, 1], F32, tag="se_dlo")
                    dhi = sm.tile([P, 1], F32, tag="se_dhi")
                    nc.vector.tensor_scalar(dlo[:], c_lo[:], float(K_TOP), None,
                                            op0=OP.subtract)
                    nc.vector.tensor_scalar(dhi[:], c_hi[:], -1.0, float(K_TOP),
                                            op0=OP.mult, op1=OP.add)
                    use_lo = sm.tile([P, 1], I32, tag="se_ul")
                    nc.vector.tensor_tensor(use_lo[:], dlo[:], dhi[:], OP.is_le)
                    t_f = sm.tile([P, 1], F32, tag="se_tf")
                    nc.vector.tensor_copy(t_f[:], hi_f[:])
                    nc.vector.copy_predicated(t_f[:], use_lo[:], lo_f[:])
                    ssel = sm.tile([P, 1], F32, tag="se_ss")
                    masked = sel2.tile([P, S], BF16, tag="jk", name="masked",
                                       bufs=2)
                    nc.vector.scalar_tensor_tensor(masked[:], Eb[:], t_f[:], Eb[:],
                                                   op0=OP.is_ge, op1=OP.mult,
                                                   accum_out=ssel[:])
                    rsel = sm.tile([P, 1], F32, tag="se_rs")
                    with nc.allow_low_precision(reason="sp renorm"):
                        nc.vector.reciprocal_approx_fast(rsel[:], ssel[:])
                    # x256 so the fp8 spT stays out of subnormal range
                    spn = sel2.tile([P, S], BF16, tag="spn", name="spn", bufs=4)
                    nc.vector.tensor_scalar(spn[:], masked[:], rsel[:], 256.0,
                                            op0=OP.mult, op1=OP.mult)
                    spn_t.append(spn)

                # ---- spT transposes (PE, after dense) + sparse PV ----
                spT = spt_pool.tile([P, SC, RS], F8, tag="spT", name="spT")
                for qi in range(QC if EN_SPARSE else 0):
                    spn = spn_t[qi]
                    for sc2 in range(SC // 4):
                        ptb = sc_tile_b()
                        for j in range(4):
                            scn = sc2 * 4 + j
                            nc.tensor.transpose(ptb[:, j * P:(j + 1) * P],
                                                spn[:, scn * P:(scn + 1) * P],
                                                identb[:])
                        nc.vector.tensor_copy(
                            spT[:, sc2 * 4:(sc2 + 1) * 4, qi * P:(qi + 1) * P],
                            ptb[:, :512].rearrange("p (a b) -> p a b", b=P))

                sp_sb = kv.tile([P, DK, RS], BF16, tag="sp_sb")
                if not EN_SPARSE:
                    nc.vector.memset(sp_sb[:], 0.0)
                for kk in range(DK if EN_SPARSE else 0):
                    pa = pspv.tile([P, 512], F32, tag="pspv", name="pa")
                    for scp in range(SC // 2):
                        nc.tensor.matmul(pa[:, :RS],
                                         Vs[:, 2 * scp:2 * scp + 2,
                                            kk * P:(kk + 1) * P],
                                         spT[:, 2 * scp:2 * scp + 2, :],
                                         start=(scp == 0), stop=(scp == SC // 2 - 1),
                                         perf_mode=DR)
                    nc.scalar.activation(sp_sb[:, kk, :], pa[:, :RS], AF.Copy,
                                         scale=1.0 / 256.0)

                # ---- out proj + gating -> u1; LN1 -> y ----
                u1 = sel.tile([P, DK, RS], F32R, tag="E", name="u1")
                for kk in range(DK):
                    ow_s = wf.tile([P, DK, P], BF16, tag="ow")
                    nc.gpsimd.dma_start(ow_s[:], ow_d.ap()[l, :, :, kk * P:(kk + 1) * P])
                    pd = hold_tile()
                    for kk2 in range(DK):
                        nc.tensor.matmul(pd[:, :RS], ow_s[:, kk2, :],
                                         attnT[:, kk2, :],
                                         start=(kk2 == 0), stop=(kk2 == DK - 1))
                    nc.vector.scalar_tensor_tensor(
                        u1[:, kk, :], pd[:, :RS], g_all[:, l:l + 1], xo[:, kk, :],
                        op0=OP.mult, op1=OP.add)
                    nc.vector.scalar_tensor_tensor(
                        u1[:, kk, :], sp_sb[:, kk, :], gm_all[:, l:l + 1],
                        u1[:, kk, :], op0=OP.mult, op1=OP.add)
                y = st.tile([P, DK, RS], F32R, tag="y")
                layernorm(u1, l, 0, y)

                # ---- FFN (bf16) ----
                yB = kv.tile([P, DK, RS], BF16, tag="yB")
                nc.vector.tensor_copy(yB[:], y[:])
                hT = spt_pool.tile([P, SC, RS], BF16, tag="hT", name="hT")
                if not EN_FFN:
                    nc.vector.memset(hT[:], 0.0)
                for fg in range(4 if EN_FFN else 0):
                    f1_s = wf.tile([P, DK, 4 * P], BF16, tag="f1")
                    nc.gpsimd.dma_start(f1_s[:], f1_d.ap()[l, :, :, fg * 512:(fg + 1) * 512])
                    for j in range(4):
                        oc = fg * 4 + j
                        ph = sc_tile()
                        for kk in range(DK):
                            nc.tensor.matmul(ph[:, :RS],
                                             f1_s[:, kk, j * P:(j + 1) * P],
                                             yB[:, kk, :],
                                             start=(kk == 0), stop=(kk == DK - 1))
                        nc.scalar.activation(hT[:, oc, :], ph[:, :RS], AF.Relu)
                u2 = sel.tile([P, DK, RS], F32R, tag="E", name="u2")
                for kk in range(DK):
                    f2_s = wf2.tile([P, FFC, P], BF16, tag="f2", bufs=2)
                    nc.gpsimd.dma_start(f2_s[:], f2_d.ap()[l, :, :, kk * P:(kk + 1) * P])
                    pf = hold_tile()
                    for oc in range(FFC):
                        nc.tensor.matmul(pf[:, :RS],
                                         f2_s[:, oc, :],
                                         hT[:, oc, :],
                                         start=(oc == 0), stop=(oc == FFC - 1))
                    nc.vector.tensor_tensor(u2[:, kk, :], pf[:, :RS], y[:, kk, :],
                                            OP.add)
                layernorm(u2, l, 1, xo)
                if dump_x:
                    nc.sync.dma_start(dbg_d.ap()[l + 1], xo[:].bitcast(F32))
                if l < L_RUN - 1:
                    xo8_cur = kv.tile([P, DK, RS], F8, tag="xo8", name="xo8n")
                    nc.vector.tensor_copy(xo8_cur[:], xo[:])
                    allgather_x8(xo8_cur)

            if os.environ.get("K_PRINT"):
                import contextlib
                with open("/tmp/prog.txt", "w") as f:
                    with contextlib.redirect_stdout(f):
                        nc.print_concise(deps=True)
            ctx2.close()
            # ---- final AllGather in bf16 for the logit matmul ----
            fin2 = ctx.enter_context(tc.tile_pool(name="fin2", bufs=2))
            xTb = fin2.tile([P, DK, S], BF16, tag="xTb", bufs=1)
            xoBf = fin2.tile([P, DK, RS], BF16, tag="xoBf", bufs=1)
            nc.vector.tensor_copy(xoBf[:], xo[:])
            ag_inbh = [dram.tile([P, 2, RS], BF16, name=f"agbi{h}")
                       for h in range(2)]
            ag_outbh = [dram.tile([4, P, 2, RS], BF16, name=f"agbo{h}")
                        for h in range(2)]
            for h in range(2):
                nc.sync.dma_start(ag_inbh[h][:], xoBf[:, 2 * h:2 * h + 2, :])
                nc.gpsimd.collective_compute(
                    "AllGather", OP.bypass,
                    replica_groups=[[0, 1, 2, 3], [4, 5, 6, 7]],
                    ins=[ag_inbh[h][:].opt()], outs=[ag_outbh[h][:].opt()])
            for h in range(2):
                for rr in range(4):
                    nc.sync.dma_start(
                        xTb[:, 2 * h:2 * h + 2, rr * RS:(rr + 1) * RS],
                        ag_outbh[h][rr])

            # ================= final projection =================
            NVC = (VSL + P - 1) // P
            for vc in range(NVC):
                grp, off = vc // 4, vc % 4
                if off == 0:
                    ftile = fin2.tile([P, DK, 4 * P], BF16, tag="fin")
                    w = min(4 * P, VSL - grp * 4 * P)
                    nc.gpsimd.dma_start(ftile[:, :, :w],
                                      fin_d.ap()[:, :, grp * 4 * P:grp * 4 * P + w])
                vw = min(P, VSL - vc * P)
                for rc in range(2):
                    pl = sc_tile()
                    for half in range(2):
                        for kk in range(DK):
                            nc.tensor.matmul(
                                pl[:vw, half * 512:(half + 1) * 512],
                                ftile[:, kk, off * P:off * P + vw],
                                xTb[:, kk,
                                    rc * 1024 + half * 512:rc * 1024 + (half + 1) * 512],
                                start=(kk == 0), stop=(kk == DK - 1))
                    lo_s = fin2.tile([P, 1024], BF16, tag="lo")
                    if (vc + rc) % 2 == 0:
                        nc.scalar.copy(lo_s[:vw, :], pl[:vw, :])
                    else:
                        nc.vector.tensor_copy(lo_s[:vw, :], pl[:vw, :])
                    nc.sync.dma_start(
                        out_d.ap()[vc * P:vc * P + vw, rc * 1024:(rc + 1) * 1024],
                        lo_s[:vw, :])

    nc.compile()
    return nc


def _prep_inputs(inputs):
    f32 = np.float32
    bf = ml_dtypes.bfloat16
    f8 = ml_dtypes.float8_e4m3
    src = np.asarray(inputs["src"]).astype(np.int64)
    emb = np.ascontiguousarray(np.asarray(inputs["emb"], f32))
    pe = np.asarray(inputs["pe"], f32)
    lam = np.asarray(inputs["lam"], f32)
    for nm in ("in_b", "out_b", "qp_b", "kp_b", "vp_b", "ff1_b", "ff2_b", "fin_b"):
        assert not np.any(np.asarray(inputs[nm])), f"nonzero bias {nm} unsupported"
    in_w = np.asarray(inputs["in_w"], f32)
    out_w = np.asarray(inputs["out_w"], f32)
    qp_w = np.asarray(inputs["qp_w"], f32)
    kp_w = np.asarray(inputs["kp_w"], f32)
    vp_w = np.asarray(inputs["vp_w"], f32)
    ff1_w = np.asarray(inputs["ff1_w"], f32)
    ff2_w = np.asarray(inputs["ff2_w"], f32)
    ln1_s = np.asarray(inputs["ln1_s"], f32)
    ln1_b = np.asarray(inputs["ln1_b"], f32)
    ln2_s = np.asarray(inputs["ln2_s"], f32)
    ln2_b = np.asarray(inputs["ln2_b"], f32)
    fin_w = np.asarray(inputs["fin_w"], f32)

    def to_pdk(w):  # [L, D, C] -> [L, P, DK, C]
        Lx, Dx, Cx = w.shape
        return np.ascontiguousarray(
            w.reshape(Lx, DK, P, Cx).transpose(0, 2, 1, 3))

    def to_f8(w):
        return np.clip(w * WSCALE, -240.0, 240.0).astype(f8)

    wl = to_f8(to_pdk(np.concatenate([
        in_w[:, 512:1024, :].transpose(0, 2, 1),
        kp_w.transpose(0, 2, 1),
        in_w[:, 0:512, :].transpose(0, 2, 1),
        qp_w.transpose(0, 2, 1),
    ], axis=2)))
    wr = to_f8(to_pdk(np.concatenate([
        in_w[:, 1024:1536, :].transpose(0, 2, 1),
        vp_w.transpose(0, 2, 1),
    ], axis=2)))
    ow = to_pdk(out_w.transpose(0, 2, 1)).astype(bf)
    f1 = to_pdk(ff1_w.transpose(0, 2, 1)).astype(bf)
    f2 = np.ascontiguousarray(
        ff2_w.transpose(0, 2, 1).reshape(L, FFC, P, D).transpose(0, 2, 1, 3)).astype(bf)
    lns = np.ascontiguousarray(
        np.stack([ln1_s, ln2_s], 1).reshape(L, 2, DK, P).transpose(0, 1, 3, 2))
    lnb = np.ascontiguousarray(
        np.stack([ln1_b, ln2_b], 1).reshape(L, 2, DK, P).transpose(0, 1, 3, 2))
    peT = np.ascontiguousarray(pe.T.reshape(DK, P, S).transpose(1, 0, 2))
    finT = np.ascontiguousarray(fin_w.T.reshape(DK, P, V).transpose(1, 0, 2))

    in_maps = []
    for c in range(NCORE):
        b, r = c // 4, c % 4
        in_maps.append({
            "emb": emb,
            "idxo": _wrap_idx(src[b, r * RS:(r + 1) * RS]),
            "peTo": np.ascontiguousarray(peT[:, :, r * RS:(r + 1) * RS]),
            "lam": lam.reshape(1, L).astype(f32),
            "wl": wl, "wr": wr, "ow": ow, "f1": f1, "f2": f2,
            "lns": lns, "lnb": lnb,
            "fin": np.ascontiguousarray(
                finT[:, :, r * VSL:(r + 1) * VSL]).astype(bf),
        })
    return in_maps


def kernel(**inputs):
    dump_x = bool(int(os.environ.get("KERNEL_DUMP_X", "0")))
    key = ("nc", dump_x)
    if key not in _CACHE:
        _CACHE[key] = build_nc(dump_x)
    nc = _CACHE[key]
    in_maps = _prep_inputs(inputs)
    trace = bool(int(os.environ.get("KERNEL_TRACE", "0")))
    res = run_bass_kernel_spmd(nc, in_maps, core_ids=list(range(NCORE)),
                               trace=trace)
    if trace:
        _CACHE["last_res"] = res
    out = np.zeros((B, S, V), np.float32)
    for c in range(NCORE):
        b, r = c // 4, c % 4
        out[b, :, r * VSL:(r + 1) * VSL] = res.results[c]["out"].T.astype(np.float32)
    if dump_x:
        _CACHE["dbg"] = [res.results[c].get("dbg") for c in range(NCORE)]
    return out


# revision 25
# speedup vs baseline: 1.8319x; 1.0202x over previous
# Trainium2 Bass kernel for nn_EnhancedEURLTransformer_87694642249910
# Sharding: 8 cores = 2 (batch) x 4 (sequence rows). Per-layer AllGather of x
# within each 4-core group (fp8 payload; final AG bf16 for the logit matmul).
# Activations transposed [D on partitions, rows free]. fp8e4 DoubleRow matmuls
# for all D-contraction projections and both PV contractions; FFN / out-proj /
# final projection stay bf16. Sparse top-K threshold found by a warm-started
# bisection seeded from per-row log-normal stats of exp(qk/8).
import os
import sys

sys.path.insert(0, "/opt/trn_rl_repo")

import math
import numpy as np
import ml_dtypes

import concourse.bass as bass
import concourse.mybir as mybir
import concourse.tile as tile
from concourse import bacc
from concourse.bass_utils import run_bass_kernel_spmd
from concourse.masks import make_identity

B, S, D, H, R, L, V, FF = 2, 2048, 512, 8, 64, 6, 32000, 2048
HD = D // H          # 64
K_TOP = 409
LN_EPS = 1e-5

F32 = mybir.dt.float32
F32R = mybir.dt.float32r
BF16 = mybir.dt.bfloat16
F8 = mybir.dt.float8e4
I16 = mybir.dt.int16
I32 = mybir.dt.int32
AF = mybir.ActivationFunctionType
OP = mybir.AluOpType
AX = mybir.AxisListType
DR = mybir.MatmulPerfMode.DoubleRow

P = 128
DK = D // P          # 4 d-chunks
SC = S // P          # 16 seq-chunks
NCORE = 8
RS = S // 4          # 512 rows per core
QC = RS // P         # 4 own-row chunks
VSL = V // 4         # 8000 vocab cols per core
FFC = FF // P        # 16
SQRT_D = math.sqrt(D)
WSCALE = 64.0        # fp8 projection-weight prescale (avoids subnormals)
LN_N = math.log(float(S))
Z_LO, Z_HI = 0.72, 0.97

N_BISECT = int(os.environ.get("K_BISECT", "5"))
L_RUN = int(os.environ.get("K_LAYERS", str(L)))
EN_SPARSE = os.environ.get("K_SPARSE", "1") == "1"
EN_DENSE = os.environ.get("K_DENSE", "1") == "1"
EN_FFN = os.environ.get("K_FFN", "1") == "1"

_CACHE = {}


def _wrap_idx(idx):
    # dma_gather index wrapping: token i -> partition i%16, col i//16
    # tile must be [128, n//16]; only partitions 0..15 are read
    n = idx.shape[0]
    return np.ascontiguousarray(
        np.tile(idx.reshape(n // 16, 16).T.astype(np.int16), (8, 1)))


def build_nc(dump_x=False):
    nc = bacc.Bacc("TRN2", target_bir_lowering=False, debug=False, num_devices=NCORE)

    emb_d = nc.dram_tensor("emb", [V, D], F32, kind="ExternalInput")
    idxo_d = nc.dram_tensor("idxo", [128, RS // 16], I16, kind="ExternalInput")
    peTo_d = nc.dram_tensor("peTo", [P, DK, RS], F32, kind="ExternalInput")
    lam_d = nc.dram_tensor("lam", [1, L], F32R, kind="ExternalInput")
    wl_d = nc.dram_tensor("wl", [L, P, DK, 1152], F8, kind="ExternalInput")
    wr_d = nc.dram_tensor("wr", [L, P, DK, 1024], F8, kind="ExternalInput")
    ow_d = nc.dram_tensor("ow", [L, P, DK, D], BF16, kind="ExternalInput")
    f1_d = nc.dram_tensor("f1", [L, P, DK, FF], BF16, kind="ExternalInput")
    f2_d = nc.dram_tensor("f2", [L, P, FFC, D], BF16, kind="ExternalInput")
    lns_d = nc.dram_tensor("lns", [L, 2, P, DK], F32, kind="ExternalInput")
    lnb_d = nc.dram_tensor("lnb", [L, 2, P, DK], F32, kind="ExternalInput")
    fin_d = nc.dram_tensor("fin", [P, DK, VSL], BF16, kind="ExternalInput")
    out_d = nc.dram_tensor("out", [VSL, S], BF16, kind="ExternalOutput")
    if dump_x:
        dbg_d = nc.dram_tensor("dbg", [L + 1, P, DK, RS], F32, kind="ExternalOutput")

    from contextlib import ExitStack
    with tile.TileContext(nc) as tc, ExitStack() as ctx:
        ep = ctx.enter_context
        st = ep(tc.tile_pool(name="state", bufs=1))
        sm = ep(tc.tile_pool(name="small", bufs=2))
        psc = ep(tc.tile_pool(name="psc", bufs=2, space="PSUM"))
        pspv = ep(tc.tile_pool(name="pspv", bufs=1, space="PSUM"))
        phold = ep(tc.tile_pool(name="phold", bufs=2, space="PSUM"))
        dram = ep(tc.tile_pool(name="dram", bufs=1, space="DRAM"))
        ctx2 = ctx.enter_context(ExitStack())
        ep2 = ctx2.enter_context
        wp = ep2(tc.tile_pool(name="wproj", bufs=1))
        wf = ep2(tc.tile_pool(name="wffn", bufs=2))
        wf2 = ep2(tc.tile_pool(name="wf2", bufs=1))
        kv = ep2(tc.tile_pool(name="kv", bufs=1))
        sel = ep2(tc.tile_pool(name="sel", bufs=1))
        sel2 = ep2(tc.tile_pool(name="sel2", bufs=1))
        spt_pool = ep2(tc.tile_pool(name="spTp", bufs=1))
        expp = ep2(tc.tile_pool(name="expp", bufs=2))
        lnp = ep2(tc.tile_pool(name="lnp", bufs=1))
        if True:
            def sc_tile():
                return psc.tile([P, 1024], F32, tag="sc", name="sc")

            def sc_tile_b():
                return psc.tile([P, 1024], BF16, tag="sc", name="scb")

            def hold_tile():
                return phold.tile([P, 512], F32, tag="hold", name="hold")

            # ------------- persistent state -------------
            xT8 = st.tile([P, DK, S], F8)           # gathered x (all rows, fp8)
            xo = st.tile([P, DK, RS], F32R)         # own rows, residual spine
            ident = st.tile([P, P], F32)
            identb = st.tile([P, P], BF16)
            ones_f32r = st.tile([P, 1], F32R)
            ones_1 = st.tile([1, P], F32R)
            g_all = st.tile([P, L], F32)
            gm_all = st.tile([P, L], F32)
            make_identity(nc, ident)
            make_identity(nc, identb)
            ones_tmp = sm.tile([P, 1], F32, tag="otmp")
            nc.vector.memset(ones_tmp[:], 1.0)
            nc.vector.tensor_copy(ones_f32r[:], ones_tmp[:])
            ones_tmp2 = sm.tile([1, P], F32, tag="otmp2")
            nc.vector.memset(ones_tmp2[:], 1.0)
            nc.vector.tensor_copy(ones_1[:], ones_tmp2[:])

            eps_t = st.tile([1, 1], F32)
            nc.vector.memset(eps_t[:], LN_EPS)
            eps_p = st.tile([P, 1], F32)
            nc.vector.memset(eps_p[:], 1e-6)
            lam_s = sm.tile([1, L], F32R, tag="lam")
            nc.sync.dma_start(lam_s[:], lam_d.ap())
            g_row = sm.tile([1, L], F32R, tag="lam")
            nc.scalar.activation(g_row[:], lam_s[:], AF.Sigmoid)
            pg = sc_tile()
            nc.tensor.matmul(pg[:, :L], ones_1[:], g_row[:], start=True, stop=True)
            nc.vector.tensor_copy(g_all[:], pg[:, :L])
            nc.vector.tensor_scalar(gm_all[:], g_all[:], -1.0, 1.0,
                                    op0=OP.mult, op1=OP.add)  # 1-g

            ag_in8h = [dram.tile([P, 2, RS], F8, name=f"agi{h}")
                       for h in range(2)]
            ag_out8h = [dram.tile([4, P, 2, RS], F8, name=f"ago{h}")
                        for h in range(2)]

            # ---------- embedding: own rows ----------
            idx_s = sm.tile([128, RS // 16], I16, tag="idx")
            nc.sync.dma_start(idx_s[:], idxo_d.ap())
            gath = sel.tile([P, QC, D], F32, tag="E", name="gath")
            nc.gpsimd.dma_gather(gath[:], emb_d.ap(), idx_s[:], RS, RS, D)
            for kk in range(DK):
                pt = sc_tile()
                for c in range(QC):
                    nc.tensor.transpose(pt[:, c * P:(c + 1) * P],
                                        gath[:, c, kk * P:(kk + 1) * P], ident[:])
                nc.vector.tensor_scalar_mul(xo[:, kk, :].bitcast(F32), pt[:, :RS],
                                            SQRT_D)
            nc.gpsimd.dma_start(xo[:].bitcast(F32), peTo_d.ap(),
                                accum_op=OP.add)

            def allgather_x8(xo8_src):
                # split by kk-halves: consumers contract pairs {0,1} first,
                # so half 0 unblocks the next layer's projections early
                for h in range(2):
                    nc.sync.dma_start(ag_in8h[h][:],
                                      xo8_src[:, 2 * h:2 * h + 2, :])
                    nc.gpsimd.collective_compute(
                        "AllGather", OP.bypass,
                        replica_groups=[[0, 1, 2, 3], [4, 5, 6, 7]],
                        ins=[ag_in8h[h][:].opt()], outs=[ag_out8h[h][:].opt()])
                for h in range(2):
                    for rr in range(4):
                        nc.sync.dma_start(
                            xT8[:, 2 * h:2 * h + 2, rr * RS:(rr + 1) * RS],
                            ag_out8h[h][rr])

            if dump_x:
                nc.sync.dma_start(dbg_d.ap()[0], xo[:].bitcast(F32))
            xo8_cur = kv.tile([P, DK, RS], F8, tag="xo8", name="xo8e")
            nc.vector.tensor_copy(xo8_cur[:], xo[:])
            allgather_x8(xo8_cur)

            rs_t = lnp.tile([128, 3 * RS], F32, name="rowscratch")
            rs2_t = lnp.tile([1, 3 * RS], F32R, name="rowscratch2")
            # ---------- layernorm: dst = LN(u) ----------
            def layernorm(u, l, which, dst):
                usq_t = []
                for kk in range(DK):
                    usq = lnp.tile([P, RS], F32R, tag="usq", name="usq")
                    nc.vector.tensor_tensor(usq, u[:, kk, :], u[:, kk, :], OP.mult)
                    usq_t.append(usq)
                psum_s = hold_tile()
                for kk in range(DK):
                    nc.tensor.matmul(psum_s[:1, :RS], ones_f32r[:], u[:, kk, :],
                                     start=(kk == 0), stop=(kk == DK - 1))
                mean = rs_t[0:1, 0:RS]
                nc.vector.tensor_scalar_mul(mean[:], psum_s[:1, :RS], 1.0 / D)
                pssq = hold_tile()
                for kk in range(DK):
                    nc.tensor.matmul(pssq[:1, :RS], ones_f32r[:], usq_t[kk][:],
                                     start=(kk == 0), stop=(kk == DK - 1))
                msq = rs_t[64:65, 0:RS]
                nc.vector.tensor_tensor(msq[:], mean[:], mean[:], OP.mult)
                var = rs_t[32:33, 0:RS]
                nc.vector.scalar_tensor_tensor(var[:], pssq[:1, :RS], 1.0 / D,
                                               msq[:], op0=OP.mult,
                                               op1=OP.subtract)
                sd = rs_t[0:1, RS:2 * RS]
                nc.scalar.activation(sd[:], var[:], AF.Sqrt, bias=eps_t[:])
                rtmp = rs_t[0:1, 2 * RS:3 * RS]
                with nc.allow_low_precision(reason="f32r istd"):
                    nc.vector.reciprocal_approx_fast(rtmp[:], sd[:])
                istd = rs2_t[0:1, 0:RS]
                nc.vector.tensor_copy(istd[:], rtmp[:])
                nistd = rs2_t[0:1, RS:2 * RS]
                nc.vector.tensor_tensor(nistd[:], mean[:].bitcast(F32R), istd[:],
                                        OP.mult)
                nc.vector.tensor_scalar_mul(nistd[:], nistd[:], -1.0)
                pA = hold_tile()
                nc.tensor.matmul(pA[:, :RS], ones_1[:], istd[:], start=True, stop=True)
                pB = hold_tile()
                nc.tensor.matmul(pB[:, :RS], ones_1[:], nistd[:], start=True, stop=True)
                scl = sm.tile([P, DK], F32, tag="ln_sc")
                bcl = sm.tile([P, DK], F32, tag="ln_bc")
                nc.sync.dma_start(scl[:], lns_d.ap()[l, which])
                nc.sync.dma_start(bcl[:], lnb_d.ap()[l, which])
                for kk in range(DK):
                    t0 = dst[:, kk, :]
                    nc.vector.tensor_tensor(t0, u[:, kk, :],
                                            pA[:, :RS].bitcast(F32R), OP.mult)
                    nc.vector.tensor_tensor(t0, t0, pB[:, :RS].bitcast(F32R), OP.add)
                    nc.vector.tensor_scalar(
                        t0, t0, scl[:, kk:kk + 1], bcl[:, kk:kk + 1],
                        op0=OP.mult, op1=OP.add)

            # ================= layers =================
            for l in range(L_RUN):
                wl_s = wp.tile([P, DK, 1152], F8, tag="wl")
                wr_s = wp.tile([P, DK, 1024], F8, tag="wr")
                nc.gpsimd.dma_start(wl_s[:], wl_d.ap()[l])
                nc.gpsimd.dma_start(wr_s[:], wr_d.ap()[l])
                xo8 = xo8_cur

                # ---- q^T, Qs^T from own rows (fp8 DoubleRow pairs) ----
                qT = kv.tile([P, DK, RS], F8, tag="qT")
                QsT = kv.tile([64, RS], F8, tag="QsT")
                for oc in range(DK):
                    pq = sc_tile()
                    for kk2 in (0, 2):
                        nc.tensor.matmul(pq[:, :RS],
                                         wl_s[:, kk2:kk2 + 2,
                                              576 + oc * P:576 + (oc + 1) * P],
                                         xo8[:, kk2:kk2 + 2, :],
                                         start=(kk2 == 0), stop=(kk2 == 2),
                                         perf_mode=DR)
                    nc.scalar.activation(qT[:, oc, :], pq[:, :RS], AF.Copy,
                                         scale=1.0 / WSCALE)
                pq = sc_tile()
                for kk2 in (0, 2):
                    nc.tensor.matmul(pq[:64, :RS], wl_s[:, kk2:kk2 + 2, 1088:1152],
                                     xo8[:, kk2:kk2 + 2, :],
                                     start=(kk2 == 0), stop=(kk2 == 2),
                                     perf_mode=DR)
                nc.scalar.activation(QsT[:], pq[:64, :RS], AF.Copy,
                                     scale=1.0 / WSCALE)

                # ---- k^T, Ks^T (full seq, fp8 DoubleRow) ----
                kT = kv.tile([P, DK, S], F8, tag="kT")
                KsT = kv.tile([64, S], F8, tag="KsT")
                for oc in range(DK):
                    for fc in range(S // 1024):
                        pk = sc_tile()
                        for hh in range(2):
                            for kk2 in (0, 2):
                                nc.tensor.matmul(
                                    pk[:, hh * 512:(hh + 1) * 512],
                                    wl_s[:, kk2:kk2 + 2, oc * P:(oc + 1) * P],
                                    xT8[:, kk2:kk2 + 2,
                                        fc * 1024 + hh * 512:fc * 1024 + (hh + 1) * 512],
                                    start=(kk2 == 0), stop=(kk2 == 2),
                                    perf_mode=DR)
                        if oc % 2 == 0:
                            nc.vector.tensor_scalar_mul(
                                kT[:, oc, fc * 1024:(fc + 1) * 1024], pk[:],
                                1.0 / WSCALE)
                        else:
                            nc.scalar.activation(
                                kT[:, oc, fc * 1024:(fc + 1) * 1024], pk[:],
                                AF.Copy, scale=1.0 / WSCALE)
                for fc in range(S // 1024):
                    pk = sc_tile()
                    for hh in range(2):
                        for kk2 in (0, 2):
                            nc.tensor.matmul(
                                pk[:64, hh * 512:(hh + 1) * 512],
                                wl_s[:, kk2:kk2 + 2, 512:576],
                                xT8[:, kk2:kk2 + 2,
                                    fc * 1024 + hh * 512:fc * 1024 + (hh + 1) * 512],
                                start=(kk2 == 0), stop=(kk2 == 2),
                                perf_mode=DR)
                    nc.scalar.activation(KsT[:, fc * 1024:(fc + 1) * 1024],
                                         pk[:64, :], AF.Copy, scale=1.0 / WSCALE)

                # ---- sparse E = exp(qk/8) + warm-start stats ----
                # per qi: Eb [P,S] bf16; accums S1 = sum E, S2 = sum E^2.
                # Stats batched [P,4] across qi so the ACT table loads once
                # per function instead of thrashing Exp/Ln/Sqrt per chunk.
                if EN_SPARSE:
                    Eb_t = []
                    s14 = sm.tile([P, 4], F32, tag="se_s14")
                    s24 = sm.tile([P, 4], F32, tag="se_s24")
                for qi in range(QC if EN_SPARSE else 0):
                    Eb = sel2.tile([P, S], BF16, tag="Eb", name="Eb", bufs=4)
                    a1 = sm.tile([P, 2], F32, tag="se_a1")
                    a2 = sm.tile([P, 2], F32, tag="se_a2")
                    for half in range(2):
                        pq_ = sc_tile()
                        nc.tensor.matmul(pq_[:, :512],
                                         QsT[:, qi * P:(qi + 1) * P],
                                         KsT[:, half * 1024:half * 1024 + 512],
                                         start=True, stop=True)
                        nc.tensor.matmul(pq_[:, 512:],
                                         QsT[:, qi * P:(qi + 1) * P],
                                         KsT[:, half * 1024 + 512:(half + 1) * 1024],
                                         start=True, stop=True)
                        nc.scalar.activation(Eb[:, half * 1024:(half + 1) * 1024],
                                             pq_[:], AF.Exp, scale=0.125,
                                             accum_out=a1[:, half:half + 1])
                        junk2 = sel2.tile([P, S], BF16, tag="jk", name="jk2",
                                          bufs=4)
                        nc.scalar.activation(junk2[:, :1024], pq_[:], AF.Exp,
                                             scale=0.25,
                                             accum_out=a2[:, half:half + 1])
                    nc.vector.tensor_tensor(s14[:, qi:qi + 1], a1[:, 0:1],
                                            a1[:, 1:2], OP.add)
                    nc.vector.tensor_tensor(s24[:, qi:qi + 1], a2[:, 0:1],
                                            a2[:, 1:2], OP.add)
                    Eb_t.append(Eb)
                if EN_SPARSE:
                    # log-normal stats -> bisect brackets lo4/hi4 [P,4]
                    L14 = sm.tile([P, 4], F32, tag="se_l14")
                    L24 = sm.tile([P, 4], F32, tag="se_l24")
                    nc.scalar.activation(L14[:], s14[:], AF.Ln)
                    nc.scalar.activation(L24[:], s24[:], AF.Ln)
                    sig2 = sm.tile([P, 4], F32, tag="se_sg2")
                    nc.vector.scalar_tensor_tensor(sig2[:], L14[:], -2.0, L24[:],
                                                   op0=OP.mult, op1=OP.add)
                    nc.vector.tensor_scalar(sig2[:], sig2[:], 1.0, LN_N,
                                            op0=OP.mult, op1=OP.add)
                    sig = sm.tile([P, 4], F32, tag="se_sg")
                    nc.scalar.activation(sig[:], sig2[:], AF.Sqrt, bias=eps_p[:])
                    mu = sm.tile([P, 4], F32, tag="se_mu")
                    nc.vector.tensor_scalar(mu[:], sig2[:], -0.5, -LN_N,
                                            op0=OP.mult, op1=OP.add)
                    nc.vector.tensor_tensor(mu[:], mu[:], L14[:], OP.add)
                    ulo = sm.tile([P, 4], F32, tag="se_ulo")
                    uhi = sm.tile([P, 4], F32, tag="se_uhi")
                    nc.vector.scalar_tensor_tensor(ulo[:], sig[:], Z_LO, mu[:],
                                                   op0=OP.mult, op1=OP.add)
                    nc.vector.scalar_tensor_tensor(uhi[:], sig[:], Z_HI, mu[:],
                                                   op0=OP.mult, op1=OP.add)
                    lo4 = sm.tile([P, 4], F32, tag="se_lo4", bufs=2)
                    hi4 = sm.tile([P, 4], F32, tag="se_hi4", bufs=2)
                    nc.scalar.activation(lo4[:], ulo[:], AF.Exp)
                    nc.scalar.activation(hi4[:], uhi[:], AF.Exp)

                # ---- v520 (ones col per head), Vs (fp8 DoubleRow) ----
                v520 = kv.tile([P, SC, 8 * 80], F8, tag="v520")
                Vs = kv.tile([P, SC, D], F8, tag="Vs")
                if l == 0:
                    nc.vector.memset(
                        v520[:].rearrange("p s (h c) -> p s h c", c=80)[:, :, :, 64:65],
                        1.0)
                for scn in range(SC):
                    pv_ = sc_tile()
                    for kk2 in (0, 2):
                        nc.tensor.matmul(pv_[:, :512],
                                         xT8[:, kk2:kk2 + 2, scn * P:(scn + 1) * P],
                                         wr_s[:, kk2:kk2 + 2, 0:512],
                                         start=(kk2 == 0), stop=(kk2 == 2),
                                         perf_mode=DR)
                    nc.vector.tensor_scalar_mul(
                        v520[:, scn, :].rearrange("p (h c) -> p h c", c=80)[:, :, :64],
                        pv_[:, :512].rearrange("p (h c) -> p h c", c=64),
                        1.0 / WSCALE)
                    pv2 = sc_tile()
                    for kk2 in (0, 2):
                        nc.tensor.matmul(pv2[:, :512],
                                         xT8[:, kk2:kk2 + 2, scn * P:(scn + 1) * P],
                                         wr_s[:, kk2:kk2 + 2, 512:1024],
                                         start=(kk2 == 0), stop=(kk2 == 2),
                                         perf_mode=DR)
                    nc.scalar.activation(Vs[:, scn, :], pv2[:, :512], AF.Copy,
                                         scale=1.0 / WSCALE)

                # ---- dense attention (scores fp8 K=64; PV fp8 DoubleRow) ----
                attnT = kv.tile([P, DK, RS], BF16, tag="attnT")
                if not EN_DENSE:
                    nc.vector.memset(attnT[:], 0.0)
                for hp in range(4 if EN_DENSE else 0):
                    pv_ps = [hold_tile(), hold_tile()]
                    for scp in range(SC // 2):
                        eTp = expp.tile([P, 2, 1024], F8, tag="eT")
                        for j in range(2):
                            scn = 2 * scp + j
                            psum_sc = sc_tile()
                            for i, h in enumerate((2 * hp, 2 * hp + 1)):
                                po = 64 * (h % 2)
                                nc.tensor.matmul(
                                    psum_sc[:, i * 512:(i + 1) * 512],
                                    kT[po:po + 64, h // 2, scn * P:(scn + 1) * P],
                                    qT[po:po + 64, h // 2, :],
                                    start=True, stop=True)
                            nc.scalar.activation(eTp[:, j, :], psum_sc[:], AF.Exp,
                                                 scale=0.125)
                        for i, h in enumerate((2 * hp, 2 * hp + 1)):
                            nc.tensor.matmul(
                                pv_ps[i][:65, :RS],
                                v520[:, 2 * scp:2 * scp + 2, h * 80:h * 80 + 65],
                                eTp[:, :, i * 512:(i + 1) * 512],
                                start=(scp == 0), stop=(scp == SC // 2 - 1),
                                perf_mode=DR)
                    for i, h in enumerate((2 * hp, 2 * hp + 1)):
                        den = rs_t[0:1, RS:2 * RS]
                        nc.scalar.copy(den[:], pv_ps[i][64:65, :RS])
                        rtmp = rs_t[0:1, 2 * RS:3 * RS]
                        with nc.allow_low_precision(reason="f32r rden"):
                            nc.vector.reciprocal_approx_fast(rtmp[:], den[:])
                        rden = rs2_t[0:1, 0:RS]
                        nc.vector.tensor_copy(rden[:], rtmp[:])
                        prb = sc_tile()
                        nc.tensor.matmul(prb[:64, :RS], ones_1[:, :64], rden[:],
                                         start=True, stop=True)
                        rb = lnp.tile([64, RS], BF16, tag="dn_rb")
                        nc.scalar.copy(rb[:], prb[:64, :RS])
                        po = 64 * (h % 2)
                        nc.vector.tensor_tensor(attnT[po:po + 64, h // 2, :],
                                                pv_ps[i][:64, :RS], rb[:], OP.mult)

                # ---- bisection on DVE (overlaps dense on PE/ACT) ----
                # iteration-major: 4 independent count passes back-to-back
                # keep the DVE pipeline full; threshold updates batched [P,4]
                spn_t = []
                if EN_SPARSE:
                    c_lo4 = sm.tile([P, 4], F32, tag="se_cl4", bufs=2)
                    c_hi4 = sm.tile([P, 4], F32, tag="se_ch4", bufs=2)
                    cnt4 = sm.tile([P, 4], F32, tag="se_cnt4", bufs=2)
                    nc.vector.memset(c_lo4[:], float(S))
                    nc.vector.memset(c_hi4[:], 1.0)
                    for it in range(N_BISECT):
                        t4 = sm.tile([P, 4], F32, tag="se_t4")
                        nc.vector.tensor_tensor(t4[:].bitcast(I32),
                                                lo4[:].bitcast(I32),
                                                hi4[:].bitcast(I32), OP.add)
                        nc.vector.tensor_scalar(t4[:].bitcast(I32),
                                                t4[:].bitcast(I32), 1, None,
                                                op0=OP.logical_shift_right)
                        junk = sel2.tile([P, S], BF16, tag="jk", name="junk",
                                         bufs=4)
                        for qi in range(QC):
                            nc.vector.tensor_scalar(
                                junk[:], Eb_t[qi][:], t4[:, qi:qi + 1], 0.0,
                                op0=OP.is_ge, op1=OP.add,
                                accum_out=cnt4[:, qi:qi + 1])
                        ge = sm.tile([P, 4], I32, tag="se_ge")
                        lt = sm.tile([P, 4], I32, tag="se_lt")
                        nc.vector.tensor_scalar(ge[:], cnt4[:], float(K_TOP), None,
                                                op0=OP.is_ge)
                        nc.vector.tensor_scalar(lt[:], cnt4[:], float(K_TOP), None,
                                                op0=OP.is_lt)
                        nc.vector.copy_predicated(lo4[:], ge[:], t4[:])
                        nc.vector.copy_predicated(c_lo4[:], ge[:], cnt4[:])
                        nc.vector.copy_predicated(hi4[:], lt[:], t4[:])
                        nc.vector.copy_predicated(c_hi4[:], lt[:], cnt4[:])
                    # pick side with count closest to K_TOP
                    dlo = sm.tile([P, 4], F32, tag="se_dlo")
                    dhi = sm.tile([P, 4], F32, tag="se_dhi")
                    nc.vector.tensor_scalar(dlo[:], c_lo4[:], float(K_TOP), None,
                                            op0=OP.subtract)
                    nc.vector.tensor_scalar(dhi[:], c_hi4[:], -1.0, float(K_TOP),
                                            op0=OP.mult, op1=OP.add)
                    use_lo = sm.tile([P, 4], I32, tag="se_ul")
                    nc.vector.tensor_tensor(use_lo[:], dlo[:], dhi[:], OP.is_le)
                    tp4 = sm.tile([P, 4], F32, tag="se_tf4")
                    nc.vector.tensor_copy(tp4[:], hi4[:])
                    nc.vector.copy_predicated(tp4[:], use_lo[:], lo4[:])
                    ssel4 = sm.tile([P, 4], F32, tag="se_ss4")
                    masked_t = []
                    for qi in range(QC):
                        masked = sel2.tile([P, S], BF16, tag="jk",
                                           name="masked", bufs=4)
                        nc.vector.scalar_tensor_tensor(
                            masked[:], Eb_t[qi][:], tp4[:, qi:qi + 1], Eb_t[qi][:],
                            op0=OP.is_ge, op1=OP.mult,
                            accum_out=ssel4[:, qi:qi + 1])
                        masked_t.append(masked)
                    rsel4 = sm.tile([P, 4], F32, tag="se_rs4")
                    with nc.allow_low_precision(reason="sp renorm"):
                        nc.vector.reciprocal_approx_fast(rsel4[:], ssel4[:])
                    for qi in range(QC):
                        # x256 so the fp8 spT stays out of subnormal range
                        spn = sel2.tile([P, S], BF16, tag="spn", name="spn",
                                        bufs=4)
                        nc.vector.tensor_scalar(spn[:], masked_t[qi][:],
                                                rsel4[:, qi:qi + 1], 256.0,
                                                op0=OP.mult, op1=OP.mult)
                        spn_t.append(spn)

                # ---- spT transposes (PE, after dense) + sparse PV ----
                spT = spt_pool.tile([P, SC, RS], F8, tag="spT", name="spT")
                for qi in range(QC if EN_SPARSE else 0):
                    spn = spn_t[qi]
                    for sc2 in range(SC // 4):
                        ptb = sc_tile_b()
                        for j in range(4):
                            scn = sc2 * 4 + j
                            nc.tensor.transpose(ptb[:, j * P:(j + 1) * P],
                                                spn[:, scn * P:(scn + 1) * P],
                                                identb[:])
                        nc.vector.tensor_copy(
                            spT[:, sc2 * 4:(sc2 + 1) * 4, qi * P:(qi + 1) * P],
                            ptb[:, :512].rearrange("p (a b) -> p a b", b=P))

                sp_sb = kv.tile([P, DK, RS], BF16, tag="sp_sb")
                if not EN_SPARSE:
                    nc.vector.memset(sp_sb[:], 0.0)
                for kk in range(DK if EN_SPARSE else 0):
                    pa = pspv.tile([P, 512], F32, tag="pspv", name="pa")
                    for scp in range(SC // 2):
                        nc.tensor.matmul(pa[:, :RS],
                                         Vs[:, 2 * scp:2 * scp + 2,
                                            kk * P:(kk + 1) * P],
                                         spT[:, 2 * scp:2 * scp + 2, :],
                                         start=(scp == 0), stop=(scp == SC // 2 - 1),
                                         perf_mode=DR)
                    nc.scalar.activation(sp_sb[:, kk, :], pa[:, :RS], AF.Copy,
                                         scale=1.0 / 256.0)

                # ---- out proj + gating -> u1; LN1 -> y ----
                u1 = sel.tile([P, DK, RS], F32R, tag="E", name="u1")
                for kk in range(DK):
                    ow_s = wf.tile([P, DK, P], BF16, tag="ow")
                    nc.gpsimd.dma_start(ow_s[:], ow_d.ap()[l, :, :, kk * P:(kk + 1) * P])
                    pd = hold_tile()
                    for kk2 in range(DK):
                        nc.tensor.matmul(pd[:, :RS], ow_s[:, kk2, :],
                                         attnT[:, kk2, :],
                                         start=(kk2 == 0), stop=(kk2 == DK - 1))
                    nc.vector.scalar_tensor_tensor(
                        u1[:, kk, :], pd[:, :RS], g_all[:, l:l + 1], xo[:, kk, :],
                        op0=OP.mult, op1=OP.add)
                    nc.vector.scalar_tensor_tensor(
                        u1[:, kk, :], sp_sb[:, kk, :], gm_all[:, l:l + 1],
                        u1[:, kk, :], op0=OP.mult, op1=OP.add)
                y = st.tile([P, DK, RS], F32R, tag="y")
                layernorm(u1, l, 0, y)

                # ---- FFN (bf16) ----
                yB = kv.tile([P, DK, RS], BF16, tag="yB")
                nc.vector.tensor_copy(yB[:], y[:])
                hT = spt_pool.tile([P, SC, RS], BF16, tag="hT", name="hT")
                if not EN_FFN:
                    nc.vector.memset(hT[:], 0.0)
                for fg in range(4 if EN_FFN else 0):
                    f1_s = wf.tile([P, DK, 4 * P], BF16, tag="f1")
                    nc.gpsimd.dma_start(f1_s[:], f1_d.ap()[l, :, :, fg * 512:(fg + 1) * 512])
                    for j in range(4):
                        oc = fg * 4 + j
                        ph = sc_tile()
                        for kk in range(DK):
                            nc.tensor.matmul(ph[:, :RS],
                                             f1_s[:, kk, j * P:(j + 1) * P],
                                             yB[:, kk, :],
                                             start=(kk == 0), stop=(kk == DK - 1))
                        nc.scalar.activation(hT[:, oc, :], ph[:, :RS], AF.Relu)
                u2 = sel.tile([P, DK, RS], F32R, tag="E", name="u2")
                for kk in range(DK):
                    f2_s = wf2.tile([P, FFC, P], BF16, tag="f2", bufs=2)
                    nc.gpsimd.dma_start(f2_s[:], f2_d.ap()[l, :, :, kk * P:(kk + 1) * P])
                    pf = hold_tile()
                    for oc in range(FFC):
                        nc.tensor.matmul(pf[:, :RS],
                                         f2_s[:, oc, :],
                                         hT[:, oc, :],
                                         start=(oc == 0), stop=(oc == FFC - 1))
                    nc.vector.tensor_tensor(u2[:, kk, :], pf[:, :RS], y[:, kk, :],
                                            OP.add)
                layernorm(u2, l, 1, xo)
                if dump_x:
                    nc.sync.dma_start(dbg_d.ap()[l + 1], xo[:].bitcast(F32))
                if l < L_RUN - 1:
                    xo8_cur = kv.tile([P, DK, RS], F8, tag="xo8", name="xo8n")
                    nc.vector.tensor_copy(xo8_cur[:], xo[:])
                    allgather_x8(xo8_cur)

            if os.environ.get("K_PRINT"):
                import contextlib
                with open("/tmp/prog.txt", "w") as f:
                    with contextlib.redirect_stdout(f):
                        nc.print_concise(deps=True)
            ctx2.close()
            # ---- final AllGather in bf16 for the logit matmul ----
            fin2 = ctx.enter_context(tc.tile_pool(name="fin2", bufs=2))
            xTb = fin2.tile([P, DK, S], BF16, tag="xTb", bufs=1)
            xoBf = fin2.tile([P, DK, RS], BF16, tag="xoBf", bufs=1)
            nc.vector.tensor_copy(xoBf[:], xo[:])
            ag_inbh = [dram.tile([P, 2, RS], BF16, name=f"agbi{h}")
                       for h in range(2)]
            ag_outbh = [dram.tile([4, P, 2, RS], BF16, name=f"agbo{h}")
                        for h in range(2)]
            for h in range(2):
                nc.sync.dma_start(ag_inbh[h][:], xoBf[:, 2 * h:2 * h + 2, :])
                nc.gpsimd.collective_compute(
                    "AllGather", OP.bypass,
                    replica_groups=[[0, 1, 2, 3], [4, 5, 6, 7]],
                    ins=[ag_inbh[h][:].opt()], outs=[ag_outbh[h][:].opt()])
            for h in range(2):
                for rr in range(4):
                    nc.sync.dma_start(
                        xTb[:, 2 * h:2 * h + 2, rr * RS:(rr + 1) * RS],
                        ag_outbh[h][rr])

            # ================= final projection =================
            NVC = (VSL + P - 1) // P
            for vc in range(NVC):
                grp, off = vc // 4, vc % 4
                if off == 0:
                    ftile = fin2.tile([P, DK, 4 * P], BF16, tag="fin")
                    w = min(4 * P, VSL - grp * 4 * P)
                    nc.gpsimd.dma_start(ftile[:, :, :w],
                                      fin_d.ap()[:, :, grp * 4 * P:grp * 4 * P + w])
                vw = min(P, VSL - vc * P)
                for rc in range(2):
                    pl = sc_tile()
                    for half in range(2):
                        for kk in range(DK):
                            nc.tensor.matmul(
                                pl[:vw, half * 512:(half + 1) * 512],
                                ftile[:, kk, off * P:off * P + vw],
                                xTb[:, kk,
                                    rc * 1024 + half * 512:rc * 1024 + (half + 1) * 512],
                                start=(kk == 0), stop=(kk == DK - 1))
                    lo_s = fin2.tile([P, 1024], BF16, tag="lo")
                    if (vc + rc) % 2 == 0:
                        nc.scalar.copy(lo_s[:vw, :], pl[:vw, :])
                    else:
                        nc.vector.tensor_copy(lo_s[:vw, :], pl[:vw, :])
                    nc.sync.dma_start(
                        out_d.ap()[vc * P:vc * P + vw, rc * 1024:(rc + 1) * 1024],
                        lo_s[:vw, :])

    nc.compile()
    return nc


def _prep_inputs(inputs):
    f32 = np.float32
    bf = ml_dtypes.bfloat16
    f8 = ml_dtypes.float8_e4m3
    src = np.asarray(inputs["src"]).astype(np.int64)
    emb = np.ascontiguousarray(np.asarray(inputs["emb"], f32))
    pe = np.asarray(inputs["pe"], f32)
    lam = np.asarray(inputs["lam"], f32)
    for nm in ("in_b", "out_b", "qp_b", "kp_b", "vp_b", "ff1_b", "ff2_b", "fin_b"):
        assert not np.any(np.asarray(inputs[nm])), f"nonzero bias {nm} unsupported"
    in_w = np.asarray(inputs["in_w"], f32)
    out_w = np.asarray(inputs["out_w"], f32)
    qp_w = np.asarray(inputs["qp_w"], f32)
    kp_w = np.asarray(inputs["kp_w"], f32)
    vp_w = np.asarray(inputs["vp_w"], f32)
    ff1_w = np.asarray(inputs["ff1_w"], f32)
    ff2_w = np.asarray(inputs["ff2_w"], f32)
    ln1_s = np.asarray(inputs["ln1_s"], f32)
    ln1_b = np.asarray(inputs["ln1_b"], f32)
    ln2_s = np.asarray(inputs["ln2_s"], f32)
    ln2_b = np.asarray(inputs["ln2_b"], f32)
    fin_w = np.asarray(inputs["fin_w"], f32)

    def to_pdk(w):  # [L, D, C] -> [L, P, DK, C]
        Lx, Dx, Cx = w.shape
        return np.ascontiguousarray(
            w.reshape(Lx, DK, P, Cx).transpose(0, 2, 1, 3))

    def to_f8(w):
        return np.clip(w * WSCALE, -240.0, 240.0).astype(f8)

    wl = to_f8(to_pdk(np.concatenate([
        in_w[:, 512:1024, :].transpose(0, 2, 1),
        kp_w.transpose(0, 2, 1),
        in_w[:, 0:512, :].transpose(0, 2, 1),
        qp_w.transpose(0, 2, 1),
    ], axis=2)))
    wr = to_f8(to_pdk(np.concatenate([
        in_w[:, 1024:1536, :].transpose(0, 2, 1),
        vp_w.transpose(0, 2, 1),
    ], axis=2)))
    ow = to_pdk(out_w.transpose(0, 2, 1)).astype(bf)
    f1 = to_pdk(ff1_w.transpose(0, 2, 1)).astype(bf)
    f2 = np.ascontiguousarray(
        ff2_w.transpose(0, 2, 1).reshape(L, FFC, P, D).transpose(0, 2, 1, 3)).astype(bf)
    lns = np.ascontiguousarray(
        np.stack([ln1_s, ln2_s], 1).reshape(L, 2, DK, P).transpose(0, 1, 3, 2))
    lnb = np.ascontiguousarray(
        np.stack([ln1_b, ln2_b], 1).reshape(L, 2, DK, P).transpose(0, 1, 3, 2))
    peT = np.ascontiguousarray(pe.T.reshape(DK, P, S).transpose(1, 0, 2))
    finT = np.ascontiguousarray(fin_w.T.reshape(DK, P, V).transpose(1, 0, 2))

    in_maps = []
    for c in range(NCORE):
        b, r = c // 4, c % 4
        in_maps.append({
            "emb": emb,
            "idxo": _wrap_idx(src[b, r * RS:(r + 1) * RS]),
            "peTo": np.ascontiguousarray(peT[:, :, r * RS:(r + 1) * RS]),
            "lam": lam.reshape(1, L).astype(f32),
            "wl": wl, "wr": wr, "ow": ow, "f1": f1, "f2": f2,
            "lns": lns, "lnb": lnb,
            "fin": np.ascontiguousarray(
                finT[:, :, r * VSL:(r + 1) * VSL]).astype(bf),
        })
    return in_maps


def kernel(**inputs):
    dump_x = bool(int(os.environ.get("KERNEL_DUMP_X", "0")))
    key = ("nc", dump_x)
    if key not in _CACHE:
        _CACHE[key] = build_nc(dump_x)
    nc = _CACHE[key]
    in_maps = _prep_inputs(inputs)
    trace = bool(int(os.environ.get("KERNEL_TRACE", "0")))
    res = run_bass_kernel_spmd(nc, in_maps, core_ids=list(range(NCORE)),
                               trace=trace)
    if trace:
        _CACHE["last_res"] = res
    out = np.zeros((B, S, V), np.float32)
    for c in range(NCORE):
        b, r = c // 4, c % 4
        out[b, :, r * VSL:(r + 1) * VSL] = res.results[c]["out"].T.astype(np.float32)
    if dump_x:
        _CACHE["dbg"] = [res.results[c].get("dbg") for c in range(NCORE)]
    return out
